# revision 1
# baseline (speedup 1.0000x reference)
"""Causal attention layer (N=8, L=2048, H=1024, E=64) on 8 TRN2 NeuronCores.

Sharding: data-parallel over batch N — one batch element per core, Q/K/V
projection weights replicated. No collectives needed.

Per-core pipeline (memory-bound problem: 24MB of q/k/v per core):
  1. q/k/v cast-loaded (f32 DRAM -> bf16 SBUF, SWDGE cast DMA) in 512-row
     chunks, then ONE flat XBAR-DMA-transpose per (tensor, chunk):
     in [128, 4096] -> out [128, 4096] whose free index m encodes
     (lp, lt, hb) = (m//32, (m%32)//8, m%8); the projection's moving-operand
     APs read it with strides [(lt:8), (lp:32)] at offset hb, which restores
     natural l-order in PSUM columns.
  2. Projections: stationary WqT/WkT/WvT [128, 64] blocks (xbar-transposed
     once), moving chunk stripes -> qpT/kpT/vpT [64, 2048] bf16, bias added
     on ScalarE during the PSUM->SBUF copy.
  3. vpT is PE-transposed to natural vp [128, 65] blocks with an appended
     ones-column (makes the context matmul accumulate softmax row-sums for
     free).
  4. Scores computed transposed: PT[j, i] = exp(scale * kp_j . qp_i), exp on
     ScalarE with the 1/sqrt(L) scale folded in; causal mask = multiplicative
     upper-triangular mask on diagonal blocks (scores are tiny: no
     max-subtraction needed).
  5. ctxT[65, i] += vp_aug[j].T @ PT[j, i] accumulated over j in PSUM;
     epilogue PE-transposes ctxT back to natural, divides by the row-sum
     column, DMAs out per stripe.
Loads are emitted k,v,q per chunk with q's last chunk hoisted before k/v's
last chunk so the deep final attention stripe starts before the load stream
finishes.
"""

import math

import numpy as np

N, L, H, E = 8, 2048, 1024, 64
NCORES = 8
CHUNK = 512  # rows per load chunk
NCHUNK = L // CHUNK  # 4
TPC = CHUNK // 128  # 128-row tiles per chunk = 4
NBLK = L // 128  # 16 j/i blocks
HB = H // 128  # 8 h-blocks

_CACHE = {}


def _build_nc(reps=1):
    from contextlib import ExitStack

    import concourse.mybir as mybir
    import concourse.tile as tile
    from concourse import bacc
    from concourse.tile_rust import add_dep_helper
    from concourse.masks import make_identity, make_upper_triangular

    f32 = mybir.dt.float32
    bf16 = mybir.dt.bfloat16
    fp8 = mybir.dt.float8e4
    AF = mybir.ActivationFunctionType
    scale = 1.0 / math.sqrt(float(L))

    nc = bacc.Bacc("TRN2", target_bir_lowering=False, debug=False)

    q_ap = nc.dram_tensor("q", [L, H], f32, kind="ExternalInput").ap()
    k_ap = nc.dram_tensor("k", [L, H], f32, kind="ExternalInput").ap()
    v_ap = nc.dram_tensor("v", [L, H], f32, kind="ExternalInput").ap()
    wq_ap = nc.dram_tensor("wq", [E, H], f32, kind="ExternalInput").ap()
    wk_ap = nc.dram_tensor("wk", [E, H], f32, kind="ExternalInput").ap()
    wv_ap = nc.dram_tensor("wv", [E, H], f32, kind="ExternalInput").ap()
    bq_ap = nc.dram_tensor("bq", [E], f32, kind="ExternalInput").ap()
    bk_ap = nc.dram_tensor("bk", [E], f32, kind="ExternalInput").ap()
    bv_ap = nc.dram_tensor("bv", [E], f32, kind="ExternalInput").ap()
    out_ap = nc.dram_tensor("out", [L, E], f32, kind="ExternalOutput").ap()

    with tile.TileContext(nc) as tc, ExitStack() as ctx:
        const = ctx.enter_context(tc.tile_pool(name="const", bufs=1))
        natp = ctx.enter_context(tc.tile_pool(name="nat", bufs=9))
        chp = ctx.enter_context(tc.tile_pool(name="ch", bufs=8))
        pTsb = ctx.enter_context(tc.tile_pool(name="pTsb", bufs=1))
        projps = ctx.enter_context(tc.tile_pool(name="projps", bufs=1, space="PSUM"))
        scps = ctx.enter_context(tc.tile_pool(name="scps", bufs=2, space="PSUM"))
        ktps = ctx.enter_context(tc.tile_pool(name="ktps", bufs=2, space="PSUM"))
        ptp = ctx.enter_context(tc.tile_pool(name="pt", bufs=3))
        ctxps = ctx.enter_context(tc.tile_pool(name="ctxps", bufs=2, space="PSUM"))
        tpsp = ctx.enter_context(tc.tile_pool(name="tps", bufs=1, space="PSUM"))
        epip = ctx.enter_context(tc.tile_pool(name="epi", bufs=4))

        # --- constants & weights: emitted via a deferred hook after the
        # first big loads so they don't block the Pool DMA queue; W is
        # sync-loaded f32 (HWDGE) and cast on VectorE, then xbar-transposed
        # to [128(h%128), 8(h//128), 64(e)] ---
        ident_f32 = const.tile([128, 128], f32)
        ident_bf16 = const.tile([128, 128], bf16)
        ident_fp8 = const.tile([128, 128], fp8)
        wtk8 = const.tile([128, HB, E], fp8)
        wtq8 = const.tile([128, HB, E], fp8)
        tri_f32 = const.tile([128, 128], f32)
        tri = const.tile([128, 128], bf16)
        wT = {}
        b_sb = {}
        wnatf = {}
        for _n in ("q", "k", "v"):
            wnatf[_n] = const.tile([E, H], f32, tag=f"wnatf_{_n}",
                                   name=f"wnatf_{_n}")
            wT[_n] = const.tile([128, HB, E], bf16, tag=f"wT_{_n}",
                                name=f"wT_{_n}")
            b_sb[_n] = const.tile([E, 1], f32, tag=f"b_{_n}",
                                  name=f"b_{_n}")

        w_xbars = []

        def emit_consts_and_weights(vaug):
            nc.vector.memset(vaug[:, :, E : E + 1], 1.0)
            make_identity(nc, ident_f32[:])
            nc.vector.tensor_copy(ident_bf16[:], ident_f32[:])
            nc.vector.tensor_copy(ident_fp8[:], ident_f32[:])
            # tri[r, c] = 1.0 where c >= r (valid: key row <= query col)
            make_upper_triangular(nc, tri_f32[:], val=1.0, diag=True)
            nc.vector.tensor_copy(tri[:], tri_f32[:])
            for name, w_ap, bias_ap in (
                ("q", wq_ap, bq_ap),
                ("k", wk_ap, bk_ap),
                ("v", wv_ap, bv_ap),
            ):
                nc.sync.dma_start(out=wnatf[name][:], in_=w_ap)
                wnat = const.tile([E, H], bf16, tag=f"wnat_{name}")
                nc.vector.tensor_copy(wnat[:], wnatf[name][:])
                w_xbars.append(
                    nc.sync.dma_start(out=wT[name][:], in_=wnat[:],
                                      transpose=True))
                nc.scalar.dma_start(out=b_sb[name][:], in_=bias_ap)
            nc.vector.tensor_copy(wtk8[:], wT["k"][:])
            nc.vector.tensor_copy(wtq8[:], wT["q"][:])

        # --- persistent projection outputs ---
        qpT = pTsb.tile([E, L], bf16, tag="qpT")
        kpT = pTsb.tile([E, L], bf16, tag="kpT")
        vpT = pTsb.tile([E, L], bf16, tag="vpT")
        vaug = pTsb.tile([128, NBLK, E + 1], bf16, tag="vaug")

        pT_of = {"q": qpT, "k": kpT, "v": vpT}
        x_ap_of = {"q": q_ap, "k": k_ap, "v": v_ap}

        out_dmas = []

        def emit_load(name, c):
            l0 = c * CHUNK
            # q and k are loaded in fp8: their quantization error only
            # reaches the softmax logits, which the 1/sqrt(L) scale
            # compresses ~45x
            dtt = fp8 if name == "k" or (name == "q" and c >= 2) else bf16
            nat = natp.tile([128, TPC, H], dtt, tag="nat")
            src = x_ap_of[name][l0 : l0 + CHUNK, :].rearrange(
                "(t p) h -> p t h", p=128
            )
            # flat out AP: bigger contiguous runs -> half the SWDGE
            # descriptors, so more loads fit the descriptor ring at once
            ld = nc.gpsimd.dma_start(
                out=nat[:].rearrange("p t h -> p (t h)"), in_=src
            )  # f32 -> bf16 cast
            return nat, ld

        def emit_tp_and_proj(name, c, nat):
            l0 = c * CHUNK
            xb = None
            pe_path = name == "k" or (name == "q" and c >= 2)
            dtt = fp8 if pe_path else bf16
            cht = chp.tile([128, TPC * H], dtt, tag="ch")
            if pe_path:
                # transpose on PE (saves serial-DMA xbar time): per (lt, hb)
                # 128x128 block transpose into PSUM, evacuate per-hb to SBUF
                # vT chunk [128, hb, l]; evac alternates ScalarE/VectorE.
                chv = cht[:].rearrange("p (hb l) -> p hb l", hb=HB, l=CHUNK)
                for hb in range(HB):
                    # fp8 transpose mode requires output element step 2
                    # (validated against the execution backend)
                    vt_ps = ktps.tile([128, 2 * CHUNK], fp8, tag="kt")
                    for t in range(TPC):
                        nc.tensor.transpose(
                            vt_ps[:, t * 256 : (t + 1) * 256 : 2],
                            nat[:, t, hb * 128 : (hb + 1) * 128],
                            ident_fp8[:],
                        )
                    vt_v = vt_ps[:, 0 : 2 * CHUNK : 2]
                    if hb % 2 == 1:
                        nc.scalar.activation(
                            chv[:, hb, :], vt_v, AF.Identity)
                    else:
                        nc.vector.tensor_copy(chv[:, hb, :], vt_v)
                rhs_of = lambda hb: chv[:, hb, :]
                w_st = wtk8 if name == "k" else wtq8
            else:
                # ONE xbar transpose per chunk: 3D out [128, TPC*HB, 128]
                # with out[a, b, c] = nat_flat[c, b*128 + a] (3D-out form
                # validated against the execution backend); free layout is
                # t*1024 + hb*128 + lp, so the projection's moving-operand AP
                # [(t: 1024), (lp: 1)] at offset hb*128 is natural l-order.
                chb = cht[:].rearrange(
                    "p (t hb lp) -> p t hb lp", t=TPC, hb=HB, lp=128
                )
                xb = nc.sync.dma_start(
                    out=cht[:].rearrange("p (b c) -> p b c", b=TPC * HB, c=128),
                    in_=nat[:].rearrange("p t h -> p (t h)"),
                    transpose=True,
                )
                rhs_of = lambda hb: chb[:, :, hb, :]
                w_st = wT[name]
            ps = projps.tile([E, CHUNK], f32, tag="projps")
            for hb in range(HB):
                nc.tensor.matmul(
                    ps[:],
                    lhsT=w_st[:, hb, :],
                    rhs=rhs_of(hb),
                    start=(hb == 0),
                    stop=(hb == HB - 1),
                )
            if name == "q":
                # VectorE is lighter-loaded than ScalarE here, and q's
                # projection gates each stripe's scores
                nc.vector.tensor_scalar_add(
                    pT_of[name][:, l0 : l0 + CHUNK], ps[:], b_sb[name][:])
            else:
                nc.scalar.activation(
                    pT_of[name][:, l0 : l0 + CHUNK], ps[:], AF.Identity,
                    bias=b_sb[name][:],
                )
            if name == "v":
                for t in range(TPC):
                    jb = c * TPC + t
                    vps = tpsp.tile([128, E + 1], bf16, tag="tps")
                    nc.tensor.transpose(
                        vps[:, :E],
                        vpT[:, jb * 128 : (jb + 1) * 128],
                        ident_bf16[:E, :E],
                    )
                    nc.vector.tensor_copy(vaug[:, jb, 0:E], vps[:, :E])
            return xb

        def begin_stripe(s):
            ctx_ps = ctxps.tile([E + 1, CHUNK], f32, tag="ctx")
            return {"s": s, "ctx": ctx_ps, "jmax": (s + 1) * TPC - 1}

        def emit_js(st, js):
            s, ctx_ps, jmax = st["s"], st["ctx"], st["jmax"]
            i0, i1 = s * CHUNK, (s + 1) * CHUNK
            # pair adjacent j's so exp runs on wider tiles (one PSUM bank)
            js = list(js)
            pairs = []
            while js:
                take = js[:1]
                w0 = i1 - max(i0, js[0] * 128)
                if len(js) > 1 and w0 + (i1 - max(i0, js[1] * 128)) <= 512:
                    take = js[:2]
                pairs.append(take)
                js = js[len(take):]
            def emit_ctx(pt, infos):
                for j, g0, w, o in infos:
                    if g0 == j * 128:  # diagonal block: causal mask
                        nc.vector.tensor_mul(
                            pt[:, o : o + 128], pt[:, o : o + 128], tri[:]
                        )
                    nc.tensor.matmul(
                        ctx_ps[:, g0 - i0 : g0 - i0 + w],
                        lhsT=vaug[:, j, :],
                        rhs=pt[:, o : o + w],
                        start=(j == 0),
                        stop=(j == jmax),
                    )

            # one-group software skew: PE's in-order queue sees
            # [scores_p, ctx_{p-1}] so it never stalls on exp_p
            pending = None
            for take in pairs:
                sc = scps.tile([128, 512], f32, tag="sc")
                pt = ptp.tile([128, 512], bf16, tag="pt")
                infos = []
                off = 0
                for j in take:
                    g0 = max(i0, j * 128)
                    w = i1 - g0
                    nc.tensor.matmul(
                        sc[:, off : off + w],
                        lhsT=kpT[:, j * 128 : (j + 1) * 128],
                        rhs=qpT[:, g0 : g0 + w],
                        start=True,
                        stop=True,
                    )
                    infos.append((j, g0, w, off))
                    off += w
                nc.scalar.activation(pt[:, 0:off], sc[:, 0:off], AF.Exp,
                                     scale=scale)
                if pending is not None:
                    emit_ctx(*pending)
                pending = (pt, infos)
            emit_ctx(*pending)

        def end_stripe(st):
            s, ctx_ps = st["s"], st["ctx"]
            i0, i1 = s * CHUNK, (s + 1) * CHUNK
            ctxsb = epip.tile([E + 1, CHUNK], f32, tag="ctxsb")
            nc.vector.tensor_copy(ctxsb[:], ctx_ps[:])
            outsb = epip.tile([128, TPC, E], f32, tag="outsb")
            for t in range(TPC):
                cps = tpsp.tile([128, E + 1], f32, tag="tps")
                nc.tensor.transpose(
                    cps[:],
                    ctxsb[:, t * 128 : (t + 1) * 128],
                    ident_f32[: E + 1, : E + 1],
                )
                rec = epip.tile([128, 1], f32, tag="rec")
                nc.vector.reciprocal(rec[:], cps[:, E : E + 1])
                nc.vector.tensor_scalar_mul(outsb[:, t, :], cps[:, 0:E], rec[:])
            dst = out_ap[i0:i1, :].rearrange("(t p) e -> p t e", p=128)
            out_dmas.append((dst, outsb))

        # Load order: (k,v,q) per chunk, q's last chunk hoisted before k/v's
        # last chunk; stripe s emitted once chunk s is fully emitted.
        load_order = []
        for c in range(NCHUNK - 1):
            load_order += [("k", c), ("v", c), ("q", c)]
        load_order += [("q", NCHUNK - 1), ("k", NCHUNK - 1), ("v", NCHUNK - 1)]

        consts_done = [False]
        for _ in range(reps):
            # Tile globally serializes every DMACopy<->DmaTranspose mode
            # transition (~2.5us dead DMA time each), so batch the stream
            # into phases: [6 loads][4 xbar transposes][6 loads][4 xbars]
            # [4 output copies] - 5 transitions instead of ~11.
            out_dmas.clear()
            # schedule: phase lists of loads; after each load's tp_proj,
            # run the stripe actions keyed to that (tensor, chunk).
            # Stripe-3's v-independent j's are emitted right after q3's
            # projection so they overlap the remaining loads; its
            # v3-dependent tail (j 12-15) comes after v3's vaug blocks.
            st_of = {}
            def s_begin(c):
                st_of[c] = begin_stripe(c)
            def s_js(c, js):
                emit_js(st_of[c], js)
            def s_end(c):
                end_stripe(st_of[c])
            phases = [
                (("k", 0), ("v", 0),
                 ("q", 0, lambda: (s_begin(0), s_js(0, range(4)), s_end(0))),
                 ("k", 1), ("v", 1),
                 ("q", 1, lambda: (s_begin(1), s_js(1, range(8)), s_end(1))),
                 ("q", 3, lambda: (s_begin(3), s_js(3, range(8)))),
                 ("q", 2, lambda: (s_begin(2), s_js(2, range(8))))),
                (("k", 2),
                 ("v", 2, lambda: (s_js(3, range(8, 12)),
                                   s_js(2, range(8, 12)), s_end(2))),
                 ("k", 3),
                 ("v", 3, lambda: (s_js(3, range(12, 16)), s_end(3)))),
            ]
            prev_last_xb = None
            for phase in phases:
                nats = []
                last_ld = None
                for item in phase:
                    n, c = item[0], item[1]
                    nat, ld = emit_load(n, c)
                    if prev_last_xb is not None:
                        add_dep_helper(
                            ld.ins, prev_last_xb.ins, sync=True,
                            reason="dma mode-phase grouping: loads after "
                                   "previous phase's transposes")
                    nats.append((item, nat))
                    last_ld = ld
                xbs = []
                if not consts_done[0]:
                    consts_done[0] = True
                    emit_consts_and_weights(vaug)
                    for wxb in w_xbars:
                        add_dep_helper(
                            wxb.ins, last_ld.ins, sync=True,
                            reason="dma mode-phase grouping: W transposes "
                                   "with first xbar group")
                for item, nat in nats:
                    n, c = item[0], item[1]
                    xb = emit_tp_and_proj(n, c, nat)
                    if xb is not None:
                        add_dep_helper(
                            xb.ins, last_ld.ins, sync=True,
                            reason="dma mode-phase grouping: transposes "
                                   "after all phase loads")
                        xbs.append(xb)
                    if len(item) > 2:
                        item[2]()
                if xbs:
                    prev_last_xb = xbs[-1]
            for dst, outsb in out_dmas:
                od = nc.scalar.dma_start(out=dst, in_=outsb[:])
                add_dep_helper(
                    od.ins, prev_last_xb.ins, sync=True,
                    reason="dma mode-phase grouping: outputs last")

    nc.compile()
    return nc


def _get_nc(reps=1):
    key = ("nc", reps)
    if key not in _CACHE:
        _CACHE[key] = _build_nc(reps)
    return _CACHE[key]


def kernel(q, k, v, key_padding_mask=None, Wq=None, bq=None, Wk=None, bk=None,
           Wv=None, bv=None):
    from concourse.bass_utils import run_bass_kernel_spmd

    nc = _get_nc()
    f = np.float32
    shared = {
        "wq": np.ascontiguousarray(Wq, dtype=f),
        "wk": np.ascontiguousarray(Wk, dtype=f),
        "wv": np.ascontiguousarray(Wv, dtype=f),
        "bq": np.ascontiguousarray(bq, dtype=f),
        "bk": np.ascontiguousarray(bk, dtype=f),
        "bv": np.ascontiguousarray(bv, dtype=f),
    }
    in_maps = []
    for n in range(NCORES):
        m = dict(shared)
        m["q"] = np.ascontiguousarray(q[n], dtype=f)
        m["k"] = np.ascontiguousarray(k[n], dtype=f)
        m["v"] = np.ascontiguousarray(v[n], dtype=f)
        in_maps.append(m)
    res = run_bass_kernel_spmd(nc, in_maps, core_ids=list(range(NCORES)))
    out = np.stack([res.results[i]["out"] for i in range(NCORES)], axis=0)
    return out.astype(np.float32)



# revision 6
# speedup vs baseline: 1.0323x; 1.0323x over previous
"""Causal attention layer (N=8, L=2048, H=1024, E=64) on 8 TRN2 NeuronCores.

Sharding: data-parallel over batch N - one batch element per core, Q/K/V
projection weights replicated. No collectives.

Per-core pipeline (all input transposes on the PE; zero DmaTranspose -> the
DMA device only carries the cast loads + outputs):
  1. q/k cast-loaded f32->fp8e4m3 (SWDGE) in 512-row chunks; v f32->bf16.
     Load order k0,q0,k1,q1,...,k3,q3,v0..v3 so scores unlock earliest.
  2. q/k chunks transposed as bf16-VIEWED fp8 PAIRS: 16 PE transposes per
     chunk (half of plain fp8) into [128(h-pair), 4(b), 512(l)] where
     partition p of block b holds h = 256b+2p+{0,1} interleaved.
  3. q/k projections as fp8 DoubleRow matmuls (contract 256 h per matmul,
     0.5 cyc/row): stationary wdr[b] = [128,2(t),64(e)] (de-interleaved W,
     built once via pair-transposes of W), moving = interleaved pair view.
     Bias added during PSUM evac; qpT/kpT stored fp8e4m3 [64, 2048].
  4. Scores ALSO run as DoubleRow at 0.5 cyc/row using stride-0 broadcast
     (both k-tiles point at the same data => logits doubled; the 1/2 is
     folded into the exp scale). exp on ScalarE; causal mask = upper-tri
     multiply on the diagonal blocks; P^T tiles persist in SBUF (bf16).
  5. v chunks: plain bf16 PE transposes (32/chunk) -> chv [128,8(hb),512],
     projection -> vpT bf16, PE-transposed to vaug [128,16,65] with a ones
     column (context matmul accumulates softmax row-sums for free).
  6. ctx^T[65, i] += vaug[j].T @ P^T[j, i] accumulated j-ordered as v chunks
     land, 2 PSUM accumulators with stripe pairing (s0,s1 then s2,s3);
     epilogue PE-transposes back, divides by row-sum, DMAs out.
"""

import math

import numpy as np

N, L, H, E = 8, 2048, 1024, 64
NCORES = 8
CHUNK = 512
NCHUNK = L // CHUNK  # 4
TPC = CHUNK // 128  # 4 l-tiles per chunk
NBLK = L // 128  # 16 j-blocks
HB = H // 128  # 8

_CACHE = {}


def _build_nc(reps=1):
    from contextlib import ExitStack

    import concourse.mybir as mybir
    import concourse.tile as tile
    from concourse import bacc
    from concourse.masks import make_identity, make_upper_triangular

    f32 = mybir.dt.float32
    bf16 = mybir.dt.bfloat16
    fp8 = mybir.dt.float8e4
    AF = mybir.ActivationFunctionType
    DR = mybir.MatmulPerfMode.DoubleRow
    # DR stride-0 scores double the logit; fold the 1/2 into the exp scale
    scale = 1.0 / (2.0 * math.sqrt(float(L)))

    nc = bacc.Bacc("TRN2", target_bir_lowering=False, debug=False)

    q_ap = nc.dram_tensor("q", [L, H], f32, kind="ExternalInput").ap()
    k_ap = nc.dram_tensor("k", [L, H], f32, kind="ExternalInput").ap()
    v_ap = nc.dram_tensor("v", [L, H], f32, kind="ExternalInput").ap()
    wq_ap = nc.dram_tensor("wq", [E, H], f32, kind="ExternalInput").ap()
    wk_ap = nc.dram_tensor("wk", [E, H], f32, kind="ExternalInput").ap()
    wv_ap = nc.dram_tensor("wv", [E, H], f32, kind="ExternalInput").ap()
    bq_ap = nc.dram_tensor("bq", [E], f32, kind="ExternalInput").ap()
    bk_ap = nc.dram_tensor("bk", [E], f32, kind="ExternalInput").ap()
    bv_ap = nc.dram_tensor("bv", [E], f32, kind="ExternalInput").ap()
    out_ap = nc.dram_tensor("out", [L, E], f32, kind="ExternalOutput").ap()

    x_ap_of = {"q": q_ap, "k": k_ap, "v": v_ap}

    with tile.TileContext(nc) as tc, ExitStack() as ctx:
        const = ctx.enter_context(tc.tile_pool(name="const", bufs=1))
        pTsb = ctx.enter_context(tc.tile_pool(name="pTsb", bufs=1))
        nat8p = ctx.enter_context(tc.tile_pool(name="nat8", bufs=3))
        natvp = ctx.enter_context(tc.tile_pool(name="natv", bufs=2))
        trqp = ctx.enter_context(tc.tile_pool(name="trq", bufs=2))
        chvp = ctx.enter_context(tc.tile_pool(name="chv", bufs=2))
        ptp = ctx.enter_context(tc.tile_pool(name="pt", bufs=1))
        epip = ctx.enter_context(tc.tile_pool(name="epi", bufs=4))
        tpps = ctx.enter_context(tc.tile_pool(name="tpps", bufs=2, space="PSUM"))
        projps = ctx.enter_context(tc.tile_pool(name="projps", bufs=1, space="PSUM"))
        scps = ctx.enter_context(tc.tile_pool(name="scps", bufs=2, space="PSUM"))
        ctxps = ctx.enter_context(tc.tile_pool(name="ctxps", bufs=2, space="PSUM"))
        smallps = ctx.enter_context(tc.tile_pool(name="smallps", bufs=1, space="PSUM"))

        # ---- persistent tiles ----
        identf = const.tile([128, 128], f32, name="identf")
        identb = const.tile([128, 128], bf16, name="identb")
        tri_f32 = const.tile([128, 128], f32, name="tri_f32")
        tri = const.tile([128, 128], bf16, name="tri")
        wf = {}
        b_sb = {}
        for nm, bias_ap in (("q", bq_ap), ("k", bk_ap), ("v", bv_ap)):
            wf[nm] = const.tile([E, H], f32, name=f"wf_{nm}")
            b_sb[nm] = const.tile([E, 1], f32, name=f"b_{nm}")
        w8 = {nm: const.tile([E, H], fp8, name=f"w8_{nm}") for nm in ("q", "k")}
        wvb = const.tile([E, H], bf16, name="wvb")
        # de-interleaved DoubleRow stationaries for q/k: [128, b, t, e]
        wdr = {nm: const.tile([128, 4, 2, E], fp8, name=f"wdr_{nm}")
               for nm in ("q", "k")}
        wTv = const.tile([128, HB, E], bf16, name="wTv")

        qpT8 = pTsb.tile([E, L], fp8, name="qpT8")
        kpT8 = pTsb.tile([E, L], fp8, name="kpT8")
        vpT = pTsb.tile([E, L], bf16, name="vpT")
        vaug = pTsb.tile([128, NBLK, E + 1], bf16, name="vaug")
        pT8_of = {"q": qpT8, "k": kpT8}

        # round-robin evac engine chooser (DVE / Act), weighted by cost
        evac_state = [0.0, 0.0]  # accumulated ns on (DVE, Act)

        def evac_copy(dst, src, dve_ns, act_ns):
            if evac_state[0] + dve_ns <= evac_state[1] + act_ns:
                evac_state[0] += dve_ns
                nc.vector.tensor_copy(dst, src)
            else:
                evac_state[1] += act_ns
                nc.scalar.activation(dst, src, AF.Identity)

        # ---- W loads (HWDGE on Act queue; tiny) ----
        def emit_w_loads():
            for nm, w_ap, bias_ap in (("q", wq_ap, bq_ap), ("k", wk_ap, bk_ap),
                                      ("v", wv_ap, bv_ap)):
                nc.scalar.dma_start(out=wf[nm][:], in_=w_ap)
                nc.scalar.dma_start(out=b_sb[nm][:], in_=bias_ap)

        def emit_consts():
            make_identity(nc, identf[:])
            nc.vector.tensor_copy(identb[:], identf[:])
            # tri[r, c] = 1 where c >= r (key row j <= query col i)
            make_upper_triangular(nc, tri_f32[:], val=1.0, diag=True)
            nc.vector.tensor_copy(tri[:], tri_f32[:])
            nc.vector.memset(vaug[:, :, E:E + 1], 1.0)

        def emit_w_prep():
            # casts
            nc.scalar.activation(w8["q"][:], wf["q"][:], AF.Identity)
            nc.scalar.activation(w8["k"][:], wf["k"][:], AF.Identity)
            nc.vector.tensor_copy(wvb[:], wf["v"][:])
            # q/k: pair-transpose W u16 view, de-interleave into wdr
            for nm in ("q", "k"):
                wu = w8[nm][:].bitcast(bf16)  # [64, 512]
                for b in range(4):
                    wps = tpps.tile([128, TPC, 128], bf16, tag="tp", name=f"wps_{nm}{b}")
                    nc.tensor.transpose(
                        wps[:, 0, 0:E], wu[:, b * 128:(b + 1) * 128],
                        identb[:E, :E])
                    nc.vector.tensor_copy(
                        wdr[nm][:, b],
                        wps[:, 0, 0:E].bitcast(fp8).rearrange(
                            "p (e t) -> p t e", t=2))
            # v: plain transposes of bf16 W
            for hb in range(HB):
                wps = tpps.tile([128, TPC, 128], bf16, tag="tp", name=f"wvps{hb}")
                nc.tensor.transpose(
                    wps[:, 0, 0:E], wvb[:, hb * 128:(hb + 1) * 128],
                    identb[:E, :E])
                nc.vector.tensor_copy(wTv[:, hb], wps[:, 0, 0:E])

        # ---- loads ----
        def emit_load(nm, c):
            l0 = c * CHUNK
            dtt = bf16 if nm == "v" else fp8
            pool = natvp if nm == "v" else nat8p
            nat = pool.tile([128, TPC, H], dtt, tag="nat", name=f"nat_{nm}{c}")
            src = x_ap_of[nm][l0:l0 + CHUNK, :].rearrange("(t p) h -> p t h", p=128)
            nc.gpsimd.dma_start(out=nat[:].rearrange("p t h -> p (t h)"), in_=src)
            return nat

        # ---- q/k chunk: pair transposes + DoubleRow projection ----
        def emit_qk_tp_proj(nm, c, nat):
            l0 = c * CHUNK
            natu = nat[:].bitcast(bf16)  # [128, TPC, 512] u16 pairs
            trq = trqp.tile([128, 4, CHUNK], bf16, tag="trq", name=f"trq_{nm}{c}")
            for lt in range(TPC):
                tps = tpps.tile([128, TPC, 128], bf16, tag="tp", name=f"tp_{nm}{c}{lt}")
                for b in range(4):
                    nc.tensor.transpose(
                        tps[:, b], natu[:, lt, b * 128:(b + 1) * 128], identb[:])
                # [128, 4, 128] -> trq[:, :, lt*128:+128]
                evac_copy(trq[:, :, lt * 128:(lt + 1) * 128], tps[:], 392, 570)
            pj = projps.tile([E, CHUNK], f32, tag="pj", name=f"pj_{nm}{c}")
            for b in range(4):
                nc.tensor.matmul(
                    pj[:],
                    lhsT=wdr[nm][:, b],
                    rhs=trq[:, b].bitcast(fp8).rearrange("p (l t) -> p t l", t=2),
                    start=(b == 0),
                    stop=(b == 3),
                    perf_mode=DR,
                )
            nc.scalar.activation(pT8_of[nm][:, l0:l0 + CHUNK], pj[:], AF.Identity,
                                 bias=b_sb[nm][:])

        # ---- scores for stripe s (all j <= 4s+3), DoubleRow stride-0 ----
        pt_info = {}  # s -> list of (pt_tile, [(j, g0, w, off)])

        def dr2(ap):
            return ap.rearrange("p (o l) -> p o l", o=1).broadcast_to(
                (ap.shape[0], 2, ap.shape[-1]))

        def emit_scores(s):
            i0, i1 = s * CHUNK, (s + 1) * CHUNK
            js = list(range(4 * s + 4))
            pairs = []
            while js:
                take = js[:1]
                w0 = i1 - max(i0, js[0] * 128)
                if len(js) > 1 and w0 + (i1 - max(i0, js[1] * 128)) <= 512:
                    take = js[:2]
                pairs.append(take)
                js = js[len(take):]
            infos_all = []
            for pi, take in enumerate(pairs):
                sc = scps.tile([128, 512], f32, tag="sc", name=f"sc_{s}_{pi}")
                pt = ptp.tile([128, 512], bf16, tag=f"pt_{s}_{pi}",
                              name=f"pt_{s}_{pi}")
                infos = []
                off = 0
                for j in take:
                    g0 = max(i0, j * 128)
                    w = i1 - g0
                    nc.tensor.matmul(
                        sc[:, off:off + w],
                        lhsT=dr2(kpT8[:, j * 128:(j + 1) * 128]),
                        rhs=dr2(qpT8[:, g0:g0 + w]),
                        start=True,
                        stop=True,
                        perf_mode=DR,
                    )
                    infos.append((j, g0, w, off))
                    off += w
                nc.scalar.activation(pt[:, 0:off], sc[:, 0:off], AF.Exp,
                                     scale=scale)
                for j, g0, w, off_ in infos:
                    if g0 == j * 128:  # diagonal block: causal mask
                        nc.vector.tensor_mul(
                            pt[:, off_:off_ + 128], pt[:, off_:off_ + 128], tri[:])
                infos_all.append((pt, infos))
            pt_info[s] = infos_all

        # ---- v chunk: plain transposes + projection + vaug ----
        def emit_v_chunk(c, nat):
            l0 = c * CHUNK
            chv = chvp.tile([128, HB, CHUNK], bf16, tag="chv", name=f"chv{c}")
            for hb in range(HB):
                vt = tpps.tile([128, TPC, 128], bf16, tag="tp", name=f"vt{c}{hb}")
                for lt in range(TPC):
                    nc.tensor.transpose(
                        vt[:, lt], nat[:, lt, hb * 128:(hb + 1) * 128], identb[:])
                evac_copy(chv[:, hb], vt[:], 392, 570)
            pj = projps.tile([E, CHUNK], f32, tag="pj", name=f"pjv{c}")
            for hb in range(HB):
                nc.tensor.matmul(
                    pj[:],
                    lhsT=wTv[:, hb],
                    rhs=chv[:, hb],
                    start=(hb == 0),
                    stop=(hb == HB - 1),
                )
            nc.vector.tensor_scalar_add(vpT[:, l0:l0 + CHUNK], pj[:], b_sb["v"][:])
            for t in range(TPC):
                jb = c * TPC + t
                sm = smallps.tile([128, E + 1], f32, tag="sm", name=f"vaugtp{jb}")
                vps = sm[:].bitcast(bf16)  # bf16 view of the f32 bank
                nc.tensor.transpose(
                    vps[:, :E], vpT[:, jb * 128:(jb + 1) * 128], identb[:E, :E])
                nc.vector.tensor_copy(vaug[:, jb, 0:E], vps[:, :E])

        # ---- ctx accumulation ----
        ctx_of = {}

        def emit_ctx(s, c):
            i0, i1 = s * CHUNK, (s + 1) * CHUNK
            jmax = 4 * s + 3
            if s not in ctx_of:
                ctx_of[s] = ctxps.tile([E + 1, CHUNK], f32, tag="ctx",
                                       name=f"ctx{s}")
            ctx_ps = ctx_of[s]
            jlo, jhi = 4 * c, min(4 * c + 3, jmax)
            for pt, infos in pt_info[s]:
                for j, g0, w, off in infos:
                    if not (jlo <= j <= jhi):
                        continue
                    nc.tensor.matmul(
                        ctx_ps[:, g0 - i0:g0 - i0 + w],
                        lhsT=vaug[:, j],
                        rhs=pt[:, off:off + w],
                        start=(j == 0),
                        stop=(j == jmax),
                    )

        out_dmas = []

        def emit_epi(s):
            i0, i1 = s * CHUNK, (s + 1) * CHUNK
            ctx_ps = ctx_of[s]
            ctxsb = epip.tile([E + 1, CHUNK], f32, tag="ctxsb", name=f"ctxsb{s}")
            nc.vector.tensor_copy(ctxsb[:], ctx_ps[:])
            outsb = epip.tile([128, TPC, E], f32, tag="outsb", name=f"outsb{s}")
            for t in range(TPC):
                cps = smallps.tile([128, E + 1], f32, tag="sm", name=f"etp{s}{t}")
                nc.tensor.transpose(
                    cps[:],
                    ctxsb[:, t * 128:(t + 1) * 128],
                    identf[:E + 1, :E + 1],
                )
                rec = epip.tile([128, 1], f32, tag="rec", name=f"rec{s}{t}")
                nc.vector.reciprocal(rec[:], cps[:, E:E + 1])
                nc.vector.tensor_scalar_mul(outsb[:, t, :], cps[:, 0:E], rec[:])
            dst = out_ap[i0:i1, :].rearrange("(t p) e -> p t e", p=128)
            nc.scalar.dma_start(out=dst, in_=outsb[:])

        # ================= emission schedule =================
        for _ in range(reps):
            evac_state[0] = evac_state[1] = 0.0
            pt_info.clear()
            ctx_of.clear()
            emit_w_loads()
            nats = {}
            nats[("k", 0)] = emit_load("k", 0)
            nats[("q", 0)] = emit_load("q", 0)
            emit_consts()  # Pool-queue consts after first two load preps
            emit_w_prep()
            for c in range(NCHUNK):
                if c > 0:
                    nats[("k", c)] = emit_load("k", c)
                    nats[("q", c)] = emit_load("q", c)
                emit_qk_tp_proj("k", c, nats[("k", c)])
                emit_qk_tp_proj("q", c, nats[("q", c)])
                emit_scores(c)
            vnats = [emit_load("v", c) for c in range(NCHUNK)]
            # v blocks with interleaved ctx; stripe pairing keeps ctxps at 2
            # bufs without in-order-queue deadlock (epi(s) is emitted before
            # any stripe that reuses its PSUM accumulator).
            for c in range(NCHUNK):
                emit_v_chunk(c, vnats[c])
                if c == 0:
                    emit_ctx(0, 0)
                    emit_epi(0)
                    emit_ctx(1, 0)
                    emit_ctx(2, 0)
                elif c == 1:
                    emit_ctx(1, 1)
                    emit_epi(1)
                    emit_ctx(2, 1)
                    emit_ctx(3, 0)
                    emit_ctx(3, 1)
                elif c == 2:
                    emit_ctx(2, 2)
                    emit_epi(2)
                    emit_ctx(3, 2)
                else:
                    emit_ctx(3, 3)
                    emit_epi(3)

    nc.compile()
    return nc


def _get_nc(reps=1):
    key = ("nc", reps)
    if key not in _CACHE:
        _CACHE[key] = _build_nc(reps)
    return _CACHE[key]


def kernel(q, k, v, key_padding_mask=None, Wq=None, bq=None, Wk=None, bk=None,
           Wv=None, bv=None):
    from concourse.bass_utils import run_bass_kernel_spmd

    nc = _get_nc()
    f = np.float32
    shared = {
        "wq": np.ascontiguousarray(Wq, dtype=f),
        "wk": np.ascontiguousarray(Wk, dtype=f),
        "wv": np.ascontiguousarray(Wv, dtype=f),
        "bq": np.ascontiguousarray(bq, dtype=f),
        "bk": np.ascontiguousarray(bk, dtype=f),
        "bv": np.ascontiguousarray(bv, dtype=f),
    }
    in_maps = []
    for n in range(NCORES):
        m = dict(shared)
        m["q"] = np.ascontiguousarray(q[n], dtype=f)
        m["k"] = np.ascontiguousarray(k[n], dtype=f)
        m["v"] = np.ascontiguousarray(v[n], dtype=f)
        in_maps.append(m)
    res = run_bass_kernel_spmd(nc, in_maps, core_ids=list(range(NCORES)))
    out = np.stack([res.results[i]["out"] for i in range(NCORES)], axis=0)
    return out.astype(np.float32)


# revision 15
# speedup vs baseline: 1.3591x; 1.3165x over previous
"""Causal attention layer (N=8, L=2048, H=1024, E=64) on 8 TRN2 NeuronCores.

Sharding: data-parallel over batch N - one batch element per core, Q/K/V
projection weights replicated. No collectives.

Per-core pipeline (all input transposes on the PE; zero DmaTranspose -> the
DMA device only carries the cast loads + outputs, ~28us):
  1. q/k cast-loaded f32->fp8e4m3 (SWDGE) in 512-row chunks; v f32->bf16.
  2. q/k chunks transposed as bf16-VIEWED fp8 PAIRS: 16 PE transposes per
     chunk (half of plain fp8) grouped 8-per-PSUM-bank, one evacuation copy
     per bank; layout [128(h-pair), 4(b), 512(l)] where partition p of
     block b holds h = 256b+2p+{0,1} interleaved.
  3. q/k projections as fp8 DoubleRow matmuls (contract 256 h per matmul,
     0.5 cyc/row): stationary wdr[b] = [128,2(t),64(e)] (de-interleaved W,
     built once via pair-transposes of W); bias added during PSUM evac;
     qpT/kpT stored fp8e4m3 [64, 2048].
  4. Scores ALSO DoubleRow at 0.5 cyc/row via stride-0 broadcast (both
     k-tiles alias the same data => logits doubled; the 1/2 is folded into
     the exp scale). exp on ScalarE; causal mask = upper-tri multiply on
     diagonal blocks; P^T tiles persist in SBUF (bf16).
  5. v chunks: plain bf16 PE transposes (32/chunk, 8-per-bank groups) ->
     chv [128,8(hb),512], projection -> vpT bf16, PE-transposed to vaug
     [128,16,65] with a ones column (ctx matmul accumulates row-sums free).
  6. ctx^T[65, i] += vaug[j].T @ P^T[j, i] accumulated j-ordered as v chunks
     land, 2 PSUM accumulators with stripe pairing (s0,s1 then s2,s3);
     epilogue PE-transposes back, divides by row-sum, DMAs out.
Engine budget: PE ~36us (bottleneck), Act = exps + proj evacs (~26us),
DVE = transpose evacs + epilogue (~28us), Pool = SWDGE descriptor gen +
late-stage copies, DMA ~28us.
"""

import math

import numpy as np

N, L, H, E = 8, 2048, 1024, 64
NCORES = 8
CHUNK = 512
NCHUNK = L // CHUNK  # 4
TPC = CHUNK // 128  # 4 l-tiles per chunk
NBLK = L // 128  # 16 j-blocks
HB = H // 128  # 8

_CACHE = {}


def _build_nc(reps=1):
    from contextlib import ExitStack

    import concourse.mybir as mybir
    import concourse.tile as tile
    from concourse import bacc
    from concourse.masks import make_identity, make_upper_triangular

    f32 = mybir.dt.float32
    bf16 = mybir.dt.bfloat16
    fp8 = mybir.dt.float8e4
    AF = mybir.ActivationFunctionType
    DR = mybir.MatmulPerfMode.DoubleRow
    # DR stride-0 scores double the logit; fold the 1/2 into the exp scale
    scale = 1.0 / (2.0 * math.sqrt(float(L)))

    nc = bacc.Bacc("TRN2", target_bir_lowering=False, debug=False)

    q_ap = nc.dram_tensor("q", [L, H], f32, kind="ExternalInput").ap()
    k_ap = nc.dram_tensor("k", [L, H], f32, kind="ExternalInput").ap()
    v_ap = nc.dram_tensor("v", [L, H], f32, kind="ExternalInput").ap()
    wq_ap = nc.dram_tensor("wq", [E, H], f32, kind="ExternalInput").ap()
    wk_ap = nc.dram_tensor("wk", [E, H], f32, kind="ExternalInput").ap()
    wv_ap = nc.dram_tensor("wv", [E, H], f32, kind="ExternalInput").ap()
    bq_ap = nc.dram_tensor("bq", [E], f32, kind="ExternalInput").ap()
    bk_ap = nc.dram_tensor("bk", [E], f32, kind="ExternalInput").ap()
    bv_ap = nc.dram_tensor("bv", [E], f32, kind="ExternalInput").ap()
    out_ap = nc.dram_tensor("out", [L, E], f32, kind="ExternalOutput").ap()

    x_ap_of = {"q": q_ap, "k": k_ap, "v": v_ap}

    with tile.TileContext(nc) as tc, ExitStack() as ctx:
        const = ctx.enter_context(tc.tile_pool(name="const", bufs=1))
        pTsb = ctx.enter_context(tc.tile_pool(name="pTsb", bufs=1))
        nat8p = ctx.enter_context(tc.tile_pool(name="nat8", bufs=6))
        natvp = ctx.enter_context(tc.tile_pool(name="natv", bufs=3))
        trqp = ctx.enter_context(tc.tile_pool(name="trq", bufs=3))
        chvp = ctx.enter_context(tc.tile_pool(name="chv", bufs=3))
        ptp = ctx.enter_context(tc.tile_pool(name="pt", bufs=1))
        epip = ctx.enter_context(tc.tile_pool(name="epi", bufs=4))
        tpps = ctx.enter_context(tc.tile_pool(name="tpps", bufs=2, space="PSUM"))
        projps = ctx.enter_context(tc.tile_pool(name="projps", bufs=1, space="PSUM"))
        scps = ctx.enter_context(tc.tile_pool(name="scps", bufs=2, space="PSUM"))
        ctxps = ctx.enter_context(tc.tile_pool(name="ctxps", bufs=2, space="PSUM"))
        smallps = ctx.enter_context(tc.tile_pool(name="smallps", bufs=1, space="PSUM"))

        # ---- persistent tiles ----
        identf = const.tile([128, 128], f32, name="identf")
        identb = const.tile([128, 128], bf16, name="identb")
        tri_f32 = const.tile([128, 128], f32, name="tri_f32")
        tri = const.tile([128, 128], bf16, name="tri")
        wf = {}
        b_sb = {}
        for nm, bias_ap in (("q", bq_ap), ("k", bk_ap), ("v", bv_ap)):
            wf[nm] = const.tile([E, H], f32, name=f"wf_{nm}")
            b_sb[nm] = const.tile([E, 1], f32, name=f"b_{nm}")
        w8 = {nm: const.tile([E, H], fp8, name=f"w8_{nm}") for nm in ("q", "k")}
        wvb = const.tile([E, H], bf16, name="wvb")
        wdr = {nm: const.tile([128, 4, 2, E], fp8, name=f"wdr_{nm}")
               for nm in ("q", "k")}
        wTv = const.tile([128, HB, E], bf16, name="wTv")

        qpT8 = pTsb.tile([E, L], fp8, name="qpT8")
        kpT8 = pTsb.tile([E, L], fp8, name="kpT8")
        vpT = pTsb.tile([E, L], bf16, name="vpT")
        vaug = pTsb.tile([128, NBLK, E + 1], bf16, name="vaug")
        pT8_of = {"q": qpT8, "k": kpT8}

        def emit_w_loads():
            for nm, w_ap, bias_ap in (("q", wq_ap, bq_ap), ("k", wk_ap, bk_ap),
                                      ("v", wv_ap, bv_ap)):
                nc.scalar.dma_start(out=wf[nm][:], in_=w_ap)
                nc.scalar.dma_start(out=b_sb[nm][:], in_=bias_ap)

        def emit_consts():
            make_identity(nc, identf[:])
            nc.vector.tensor_copy(identb[:], identf[:])
            # tri[r, c] = 1 where c >= r (key row j <= query col i)
            make_upper_triangular(nc, tri_f32[:], val=1.0, diag=True)
            nc.vector.tensor_copy(tri[:], tri_f32[:])
            nc.vector.memset(vaug[:, :, E:E + 1], 1.0)

        def emit_w_prep():
            nc.vector.tensor_copy(w8["q"][:], wf["q"][:])
            nc.vector.tensor_copy(w8["k"][:], wf["k"][:])
            nc.vector.tensor_copy(wvb[:], wf["v"][:])
            # q/k: pair-transpose W's u16 view, de-interleave into wdr
            for nm in ("q", "k"):
                wu = w8[nm][:].bitcast(bf16)  # [64, 512]
                wps = tpps.tile([128, 2, TPC, 128], bf16, tag="tp",
                                name=f"wps_{nm}")
                for b in range(4):
                    nc.tensor.transpose(
                        wps[:, 0, b, 0:E], wu[:, b * 128:(b + 1) * 128],
                        identb[:E, :E])
                nc.vector.tensor_copy(
                    wdr[nm][:],
                    wps[:, 0, :, 0:E].bitcast(fp8).rearrange(
                        "p b (e t) -> p b t e", t=2))
            # v: plain transposes of bf16 W
            wvps = {}
            for g in range(2):
                wps = tpps.tile([128, 2, TPC, 128], bf16, tag="tp",
                                name=f"wvps{g}")
                for i in range(4):
                    hb = g * 4 + i
                    nc.tensor.transpose(
                        wps[:, i // 2, i % 2, 0:E],
                        wvb[:, hb * 128:(hb + 1) * 128], identb[:E, :E])
                nc.vector.tensor_copy(
                    wTv[:, g * 4:(g + 1) * 4].rearrange(
                        "p (a b) e -> p a b e", a=2),
                    wps[:, :, 0:2, 0:E])

        def emit_load(nm, c):
            l0 = c * CHUNK
            dtt = bf16 if nm == "v" else fp8
            pool = natvp if nm == "v" else nat8p
            nat = pool.tile([128, TPC, H], dtt, tag="nat", name=f"nat_{nm}{c}")
            src = x_ap_of[nm][l0:l0 + CHUNK, :].rearrange("(t p) h -> p t h", p=128)
            nc.gpsimd.dma_start(out=nat[:].rearrange("p t h -> p (t h)"), in_=src)
            return nat

        # ---- q/k chunk: pair transposes (2 groups of 8) + DR projection ----
        def emit_qk_tp_proj(nm, c, nat):
            l0 = c * CHUNK
            natu = nat[:].bitcast(bf16)  # [128, TPC, 512] u16 pairs
            trq = trqp.tile([128, 4, CHUNK], bf16, tag="trq", name=f"trq_{nm}{c}")
            for g in range(2):  # lt in {2g, 2g+1}
                tps = tpps.tile([128, 2, 4, 128], bf16, tag="tp",
                                name=f"tp_{nm}{c}{g}")
                for i in range(2):
                    lt = 2 * g + i
                    for b in range(4):
                        nc.tensor.transpose(
                            tps[:, i, b], natu[:, lt, b * 128:(b + 1) * 128],
                            identb[:])
                # [128, (lt2, b, l)] -> trq[:, b, (2g+i)*128 + l]
                nc.vector.tensor_copy(
                    trq[:, :, g * 256:(g + 1) * 256].rearrange(
                        "p b (i l) -> p i b l", i=2),
                    tps[:])
            pj = projps.tile([E, CHUNK], f32, tag="pj", name=f"pj_{nm}{c}")
            for b in range(4):
                nc.tensor.matmul(
                    pj[:],
                    lhsT=wdr[nm][:, b],
                    rhs=trq[:, b].bitcast(fp8).rearrange("p (l t) -> p t l", t=2),
                    start=(b == 0),
                    stop=(b == 3),
                    perf_mode=DR,
                )
            nc.scalar.activation(pT8_of[nm][:, l0:l0 + CHUNK], pj[:], AF.Identity,
                                 bias=b_sb[nm][:])

        # ---- scores for stripe s (all j <= 4s+3), DoubleRow stride-0 ----
        pt_info = {}

        def dr2(ap):
            return ap.rearrange("p (o l) -> p o l", o=1).broadcast_to(
                (ap.shape[0], 2, ap.shape[-1]))

        def emit_scores(s):
            i0, i1 = s * CHUNK, (s + 1) * CHUNK
            js = list(range(4 * s + 4))
            pairs = []
            while js:
                take = js[:1]
                w0 = i1 - max(i0, js[0] * 128)
                if len(js) > 1 and w0 + (i1 - max(i0, js[1] * 128)) <= 512:
                    take = js[:2]
                pairs.append(take)
                js = js[len(take):]
            infos_all = []
            for pi, take in enumerate(pairs):
                sc = scps.tile([128, 512], f32, tag="sc", name=f"sc_{s}_{pi}")
                pt = ptp.tile([128, 512], bf16, tag=f"pt_{s}_{pi}",
                              name=f"pt_{s}_{pi}")
                infos = []
                off = 0
                for j in take:
                    g0 = max(i0, j * 128)
                    w = i1 - g0
                    nc.tensor.matmul(
                        sc[:, off:off + w],
                        lhsT=dr2(kpT8[:, j * 128:(j + 1) * 128]),
                        rhs=dr2(qpT8[:, g0:g0 + w]),
                        start=True,
                        stop=True,
                        perf_mode=DR,
                    )
                    infos.append((j, g0, w, off))
                    off += w
                nc.scalar.activation(pt[:, 0:off], sc[:, 0:off], AF.Exp,
                                     scale=scale)
                for j, g0, w, off_ in infos:
                    if g0 == j * 128:  # diagonal block: causal mask
                        nc.vector.tensor_mul(
                            pt[:, off_:off_ + 128], pt[:, off_:off_ + 128], tri[:])
                infos_all.append((pt, infos))
            pt_info[s] = infos_all

        # ---- v chunk: plain transposes (4 groups of 8) + proj + vaug ----
        def emit_v_chunk(c, nat):
            l0 = c * CHUNK
            late = c >= 2  # Pool's SWDGE preps are done by then
            chv = chvp.tile([128, HB, CHUNK], bf16, tag="chv", name=f"chv{c}")
            for g in range(4):  # hb in {2g, 2g+1}
                vt = tpps.tile([128, 2, TPC, 128], bf16, tag="tp",
                               name=f"vt{c}{g}")
                for i in range(2):
                    hb = 2 * g + i
                    for lt in range(TPC):
                        nc.tensor.transpose(
                            vt[:, i, lt], nat[:, lt, hb * 128:(hb + 1) * 128],
                            identb[:])
                if late and g % 2 == 1:
                    nc.scalar.activation(chv[:, 2 * g:2 * g + 2], vt[:],
                                         AF.Identity)
                else:
                    nc.vector.tensor_copy(chv[:, 2 * g:2 * g + 2], vt[:])
            pj = projps.tile([E, CHUNK], f32, tag="pj", name=f"pjv{c}")
            for hb in range(HB):
                nc.tensor.matmul(
                    pj[:],
                    lhsT=wTv[:, hb],
                    rhs=chv[:, hb],
                    start=(hb == 0),
                    stop=(hb == HB - 1),
                )
            nc.vector.tensor_scalar_add(vpT[:, l0:l0 + CHUNK], pj[:], b_sb["v"][:])
            for t in range(TPC):
                jb = c * TPC + t
                sm = smallps.tile([128, E + 1], f32, tag="sm", name=f"vaugtp{jb}")
                vps = sm[:].bitcast(bf16)
                nc.tensor.transpose(
                    vps[:, :E], vpT[:, jb * 128:(jb + 1) * 128], identb[:E, :E])
                nc.vector.tensor_copy(vaug[:, jb, 0:E], vps[:, :E])

        # ---- ctx accumulation ----
        ctx_of = {}

        def emit_ctx(s, c):
            i0 = s * CHUNK
            jmax = 4 * s + 3
            if s not in ctx_of:
                ctx_of[s] = ctxps.tile([E + 1, CHUNK], f32, tag="ctx",
                                       name=f"ctx{s}")
            ctx_ps = ctx_of[s]
            jlo, jhi = 4 * c, min(4 * c + 3, jmax)
            for pt, infos in pt_info[s]:
                for j, g0, w, off in infos:
                    if not (jlo <= j <= jhi):
                        continue
                    nc.tensor.matmul(
                        ctx_ps[:, g0 - i0:g0 - i0 + w],
                        lhsT=vaug[:, j],
                        rhs=pt[:, off:off + w],
                        start=(j == 0),
                        stop=(j == jmax),
                    )

        def emit_epi(s):
            i0, i1 = s * CHUNK, (s + 1) * CHUNK
            late = s >= 2
            ctx_ps = ctx_of[s]
            ctxsb = epip.tile([E + 1, CHUNK], f32, tag="ctxsb", name=f"ctxsb{s}")
            nc.vector.tensor_copy(ctxsb[:], ctx_ps[:])
            outsb = epip.tile([128, TPC, E], f32, tag="outsb", name=f"outsb{s}")
            for t in range(TPC):
                cps = smallps.tile([128, E + 1], f32, tag="sm", name=f"etp{s}{t}")
                nc.tensor.transpose(
                    cps[:],
                    ctxsb[:, t * 128:(t + 1) * 128],
                    identf[:E + 1, :E + 1],
                )
                rec = epip.tile([128, 1], f32, tag="rec", name=f"rec{s}{t}")
                nc.vector.reciprocal(rec[:], cps[:, E:E + 1])
                nc.vector.tensor_scalar_mul(outsb[:, t, :], cps[:, 0:E], rec[:])
            dst = out_ap[i0:i1, :].rearrange("(t p) e -> p t e", p=128)
            nc.scalar.dma_start(out=dst, in_=outsb[:])

        # ================= emission schedule =================
        for _ in range(reps):
            pt_info.clear()
            ctx_of.clear()
            emit_w_loads()
            nats = {}
            nats[("k", 0)] = emit_load("k", 0)
            nats[("q", 0)] = emit_load("q", 0)
            emit_consts()  # Pool-queue consts after first two load preps
            emit_w_prep()
            for c in range(NCHUNK):
                if c > 0:
                    nats[("k", c)] = emit_load("k", c)
                    nats[("q", c)] = emit_load("q", c)
                emit_qk_tp_proj("k", c, nats[("k", c)])
                emit_qk_tp_proj("q", c, nats[("q", c)])
                emit_scores(c)
            vnats = [emit_load("v", c) for c in range(NCHUNK)]
            # v blocks with interleaved ctx; stripe pairing keeps ctxps at 2
            # bufs without in-order-queue deadlock (epi(s) is emitted before
            # any stripe that reuses its PSUM accumulator).
            for c in range(NCHUNK):
                emit_v_chunk(c, vnats[c])
                if c == 0:
                    emit_ctx(0, 0)
                    emit_epi(0)
                    emit_ctx(1, 0)
                    emit_ctx(2, 0)
                elif c == 1:
                    emit_ctx(1, 1)
                    emit_epi(1)
                    emit_ctx(2, 1)
                    emit_ctx(3, 0)
                    emit_ctx(3, 1)
                elif c == 2:
                    emit_ctx(2, 2)
                    emit_epi(2)
                    emit_ctx(3, 2)
                else:
                    emit_ctx(3, 3)
                    emit_epi(3)

    nc.compile()
    return nc


def _get_nc(reps=1):
    key = ("nc", reps)
    if key not in _CACHE:
        _CACHE[key] = _build_nc(reps)
    return _CACHE[key]


def kernel(q, k, v, key_padding_mask=None, Wq=None, bq=None, Wk=None, bk=None,
           Wv=None, bv=None):
    from concourse.bass_utils import run_bass_kernel_spmd

    nc = _get_nc()
    f = np.float32
    shared = {
        "wq": np.ascontiguousarray(Wq, dtype=f),
        "wk": np.ascontiguousarray(Wk, dtype=f),
        "wv": np.ascontiguousarray(Wv, dtype=f),
        "bq": np.ascontiguousarray(bq, dtype=f),
        "bk": np.ascontiguousarray(bk, dtype=f),
        "bv": np.ascontiguousarray(bv, dtype=f),
    }
    in_maps = []
    for n in range(NCORES):
        m = dict(shared)
        m["q"] = np.ascontiguousarray(q[n], dtype=f)
        m["k"] = np.ascontiguousarray(k[n], dtype=f)
        m["v"] = np.ascontiguousarray(v[n], dtype=f)
        in_maps.append(m)
    res = run_bass_kernel_spmd(nc, in_maps, core_ids=list(range(NCORES)))
    out = np.stack([res.results[i]["out"] for i in range(NCORES)], axis=0)
    return out.astype(np.float32)


# revision 18
# speedup vs baseline: 1.4522x; 1.0685x over previous
"""Causal attention layer (N=8, L=2048, H=1024, E=64) on 8 TRN2 NeuronCores.

Sharding: data-parallel over batch N - one batch element per core, Q/K/V
projection weights replicated. No collectives.

Per-core pipeline (all input transposes on the PE; zero DmaTranspose -> the
DMA device only carries the cast loads + outputs, ~28us):
  1. q/k cast-loaded f32->fp8e4m3 (SWDGE) in 512-row chunks; v f32->bf16.
  2. q/k chunks transposed as bf16-VIEWED fp8 PAIRS: 16 PE transposes per
     chunk (half of plain fp8) grouped 8-per-PSUM-bank, one evacuation copy
     per bank; layout [128(h-pair), 4(b), 512(l)] where partition p of
     block b holds h = 256b+2p+{0,1} interleaved.
  3. q/k projections as fp8 DoubleRow matmuls (contract 256 h per matmul,
     0.5 cyc/row): stationary wdr[b] = [128,2(t),64(e)] (de-interleaved W,
     built once via pair-transposes of W); bias added during PSUM evac;
     qpT/kpT stored fp8e4m3 [64, 2048].
  4. Scores ALSO DoubleRow at 0.5 cyc/row via stride-0 broadcast (both
     k-tiles alias the same data => logits doubled; the 1/2 is folded into
     the exp scale). exp on ScalarE; causal mask = upper-tri multiply on
     diagonal blocks; P^T tiles persist in SBUF (bf16).
  5. v chunks: plain bf16 PE transposes (32/chunk, 8-per-bank groups) ->
     chv [128,8(hb),512], projection -> vpT bf16, PE-transposed to vaug
     [128,16,65] with a ones column (ctx matmul accumulates row-sums free).
  6. ctx^T[65, i] += vaug[j].T @ P^T[j, i] accumulated j-ordered as v chunks
     land, 2 PSUM accumulators with stripe pairing (s0,s1 then s2,s3);
     epilogue PE-transposes back, divides by row-sum, DMAs out.
Engine budget: PE ~36us (bottleneck), Act = exps + proj evacs (~26us),
DVE = transpose evacs + epilogue (~28us), Pool = SWDGE descriptor gen +
late-stage copies, DMA ~28us.
"""

import math

import numpy as np

N, L, H, E = 8, 2048, 1024, 64
NCORES = 8
CHUNK = 512
NCHUNK = L // CHUNK  # 4
TPC = CHUNK // 128  # 4 l-tiles per chunk
NBLK = L // 128  # 16 j-blocks
HB = H // 128  # 8

_CACHE = {}


def _build_nc(reps=1):
    from contextlib import ExitStack

    import concourse.mybir as mybir
    import concourse.tile as tile
    from concourse import bacc
    from concourse.masks import make_identity, make_upper_triangular

    f32 = mybir.dt.float32
    bf16 = mybir.dt.bfloat16
    fp8 = mybir.dt.float8e4
    AF = mybir.ActivationFunctionType
    DR = mybir.MatmulPerfMode.DoubleRow
    # DR stride-0 scores double the logit; fold the 1/2 into the exp scale
    scale = 1.0 / (2.0 * math.sqrt(float(L)))

    nc = bacc.Bacc("TRN2", target_bir_lowering=False, debug=False)

    q_ap = nc.dram_tensor("q", [L, H], f32, kind="ExternalInput").ap()
    k_ap = nc.dram_tensor("k", [L, H], f32, kind="ExternalInput").ap()
    v_ap = nc.dram_tensor("v", [L, H], f32, kind="ExternalInput").ap()
    wq_ap = nc.dram_tensor("wq", [E, H], f32, kind="ExternalInput").ap()
    wk_ap = nc.dram_tensor("wk", [E, H], f32, kind="ExternalInput").ap()
    wv_ap = nc.dram_tensor("wv", [E, H], f32, kind="ExternalInput").ap()
    bq_ap = nc.dram_tensor("bq", [E], f32, kind="ExternalInput").ap()
    bk_ap = nc.dram_tensor("bk", [E], f32, kind="ExternalInput").ap()
    bv_ap = nc.dram_tensor("bv", [E], f32, kind="ExternalInput").ap()
    out_ap = nc.dram_tensor("out", [L, E], f32, kind="ExternalOutput").ap()

    x_ap_of = {"q": q_ap, "k": k_ap, "v": v_ap}

    with tile.TileContext(nc) as tc, ExitStack() as ctx:
        const = ctx.enter_context(tc.tile_pool(name="const", bufs=1))
        pTsb = ctx.enter_context(tc.tile_pool(name="pTsb", bufs=1))
        nat8p = ctx.enter_context(tc.tile_pool(name="nat8", bufs=6))
        natvp = ctx.enter_context(tc.tile_pool(name="natv", bufs=4))
        trqp = ctx.enter_context(tc.tile_pool(name="trq", bufs=3))
        chvp = ctx.enter_context(tc.tile_pool(name="chv", bufs=3))
        ptp = ctx.enter_context(tc.tile_pool(name="pt", bufs=1))
        epip = ctx.enter_context(tc.tile_pool(name="epi", bufs=4))
        tpps = ctx.enter_context(tc.tile_pool(name="tpps", bufs=2, space="PSUM"))
        projps = ctx.enter_context(tc.tile_pool(name="projps", bufs=1, space="PSUM"))
        scps = ctx.enter_context(tc.tile_pool(name="scps", bufs=2, space="PSUM"))
        ctxps = ctx.enter_context(tc.tile_pool(name="ctxps", bufs=2, space="PSUM"))
        smallps = ctx.enter_context(tc.tile_pool(name="smallps", bufs=1, space="PSUM"))

        # ---- persistent tiles ----
        identf = const.tile([128, 128], f32, name="identf")
        identb = const.tile([128, 128], bf16, name="identb")
        tri_f32 = const.tile([128, 128], f32, name="tri_f32")
        tri = const.tile([128, 128], bf16, name="tri")
        wf = {}
        b_sb = {}
        for nm, bias_ap in (("q", bq_ap), ("k", bk_ap), ("v", bv_ap)):
            wf[nm] = const.tile([E, H], f32, name=f"wf_{nm}")
            b_sb[nm] = const.tile([E, 1], f32, name=f"b_{nm}")
        w8 = {nm: const.tile([E, H], fp8, name=f"w8_{nm}") for nm in ("q", "k")}
        wvb = const.tile([E, H], bf16, name="wvb")
        wdr = {nm: const.tile([128, 4, 2, E], fp8, name=f"wdr_{nm}")
               for nm in ("q", "k")}
        wTv = const.tile([128, HB, E], bf16, name="wTv")

        qpT8 = pTsb.tile([E, L], fp8, name="qpT8")
        kpT8 = pTsb.tile([E, L], fp8, name="kpT8")
        vpT = pTsb.tile([E, L], bf16, name="vpT")
        vaug = pTsb.tile([128, NBLK, E + 1], bf16, name="vaug")
        pT8_of = {"q": qpT8, "k": kpT8}

        def emit_w_loads():
            for nm, w_ap, bias_ap in (("q", wq_ap, bq_ap), ("k", wk_ap, bk_ap),
                                      ("v", wv_ap, bv_ap)):
                nc.scalar.dma_start(out=wf[nm][:], in_=w_ap)
                nc.scalar.dma_start(out=b_sb[nm][:], in_=bias_ap)

        def emit_consts():
            make_identity(nc, identf[:])
            nc.vector.tensor_copy(identb[:], identf[:])
            # tri[r, c] = 1 where c >= r (key row j <= query col i)
            make_upper_triangular(nc, tri_f32[:], val=1.0, diag=True)
            nc.vector.tensor_copy(tri[:], tri_f32[:])
            nc.vector.memset(vaug[:, :, E:E + 1], 1.0)

        def emit_w_prep():
            nc.vector.tensor_copy(w8["q"][:], wf["q"][:])
            nc.vector.tensor_copy(w8["k"][:], wf["k"][:])
            nc.vector.tensor_copy(wvb[:], wf["v"][:])
            # q/k: pair-transpose W's u16 view, de-interleave into wdr
            for nm in ("q", "k"):
                wu = w8[nm][:].bitcast(bf16)  # [64, 512]
                wps = tpps.tile([128, 2, TPC, 128], bf16, tag="tp",
                                name=f"wps_{nm}")
                for b in range(4):
                    nc.tensor.transpose(
                        wps[:, 0, b, 0:E], wu[:, b * 128:(b + 1) * 128],
                        identb[:E, :E])
                nc.vector.tensor_copy(
                    wdr[nm][:],
                    wps[:, 0, :, 0:E].bitcast(fp8).rearrange(
                        "p b (e t) -> p b t e", t=2))
            # v: plain transposes of bf16 W
            wvps = {}
            for g in range(2):
                wps = tpps.tile([128, 2, TPC, 128], bf16, tag="tp",
                                name=f"wvps{g}")
                for i in range(4):
                    hb = g * 4 + i
                    nc.tensor.transpose(
                        wps[:, i // 2, i % 2, 0:E],
                        wvb[:, hb * 128:(hb + 1) * 128], identb[:E, :E])
                nc.vector.tensor_copy(
                    wTv[:, g * 4:(g + 1) * 4].rearrange(
                        "p (a b) e -> p a b e", a=2),
                    wps[:, :, 0:2, 0:E])

        def emit_load(nm, c):
            l0 = c * CHUNK
            dtt = bf16 if nm == "v" else fp8
            pool = natvp if nm == "v" else nat8p
            nat = pool.tile([128, TPC, H], dtt, tag="nat", name=f"nat_{nm}{c}")
            src = x_ap_of[nm][l0:l0 + CHUNK, :].rearrange("(t p) h -> p t h", p=128)
            nc.gpsimd.dma_start(out=nat[:].rearrange("p t h -> p (t h)"), in_=src)
            return nat

        # ---- q/k chunk: pair transposes (2 groups of 8) + DR projection ----
        def emit_qk_tp_proj(nm, c, nat):
            l0 = c * CHUNK
            natu = nat[:].bitcast(bf16)  # [128, TPC, 512] u16 pairs
            trq = trqp.tile([128, 4, CHUNK], bf16, tag="trq", name=f"trq_{nm}{c}")
            for g in range(2):  # lt in {2g, 2g+1}
                tps = tpps.tile([128, 2, 4, 128], bf16, tag="tp",
                                name=f"tp_{nm}{c}{g}")
                for i in range(2):
                    lt = 2 * g + i
                    for b in range(4):
                        nc.tensor.transpose(
                            tps[:, i, b], natu[:, lt, b * 128:(b + 1) * 128],
                            identb[:])
                # [128, (lt2, b, l)] -> trq[:, b, (2g+i)*128 + l]
                nc.vector.tensor_copy(
                    trq[:, :, g * 256:(g + 1) * 256].rearrange(
                        "p b (i l) -> p i b l", i=2),
                    tps[:])
            pj = projps.tile([E, CHUNK], f32, tag="pj", name=f"pj_{nm}{c}")
            for b in range(4):
                nc.tensor.matmul(
                    pj[:],
                    lhsT=wdr[nm][:, b],
                    rhs=trq[:, b].bitcast(fp8).rearrange("p (l t) -> p t l", t=2),
                    start=(b == 0),
                    stop=(b == 3),
                    perf_mode=DR,
                )
            nc.scalar.activation(pT8_of[nm][:, l0:l0 + CHUNK], pj[:], AF.Identity,
                                 bias=b_sb[nm][:])

        # ---- scores for stripe s (all j <= 4s+3), DoubleRow stride-0 ----
        pt_info = {}

        def dr2(ap):
            return ap.rearrange("p (o l) -> p o l", o=1).broadcast_to(
                (ap.shape[0], 2, ap.shape[-1]))

        def emit_scores(s):
            i0, i1 = s * CHUNK, (s + 1) * CHUNK
            js = list(range(4 * s + 4))
            pairs = []
            while js:
                take = js[:1]
                w0 = i1 - max(i0, js[0] * 128)
                if len(js) > 1 and w0 + (i1 - max(i0, js[1] * 128)) <= 512:
                    take = js[:2]
                pairs.append(take)
                js = js[len(take):]
            infos_all = []
            for pi, take in enumerate(pairs):
                sc = scps.tile([128, 512], f32, tag="sc", name=f"sc_{s}_{pi}")
                pt = ptp.tile([128, 512], bf16, tag=f"pt_{s}_{pi}",
                              name=f"pt_{s}_{pi}")
                infos = []
                off = 0
                for j in take:
                    g0 = max(i0, j * 128)
                    w = i1 - g0
                    nc.tensor.matmul(
                        sc[:, off:off + w],
                        lhsT=dr2(kpT8[:, j * 128:(j + 1) * 128]),
                        rhs=dr2(qpT8[:, g0:g0 + w]),
                        start=True,
                        stop=True,
                        perf_mode=DR,
                    )
                    infos.append((j, g0, w, off))
                    off += w
                nc.scalar.activation(pt[:, 0:off], sc[:, 0:off], AF.Exp,
                                     scale=scale)
                for j, g0, w, off_ in infos:
                    if g0 == j * 128:  # diagonal block: causal mask
                        nc.vector.tensor_mul(
                            pt[:, off_:off_ + 128], pt[:, off_:off_ + 128], tri[:])
                infos_all.append((pt, infos))
            pt_info[s] = infos_all

        # ---- v chunk: plain transposes (4 groups of 8) + proj + vaug ----
        def emit_v_chunk(c, nat):
            l0 = c * CHUNK
            late = c >= 2  # Pool's SWDGE preps are done by then
            chv = chvp.tile([128, HB, CHUNK], bf16, tag="chv", name=f"chv{c}")
            for g in range(4):  # hb in {2g, 2g+1}
                vt = tpps.tile([128, 2, TPC, 128], bf16, tag="tp",
                               name=f"vt{c}{g}")
                for i in range(2):
                    hb = 2 * g + i
                    for lt in range(TPC):
                        nc.tensor.transpose(
                            vt[:, i, lt], nat[:, lt, hb * 128:(hb + 1) * 128],
                            identb[:])
                if late and g % 2 == 1:
                    nc.scalar.activation(chv[:, 2 * g:2 * g + 2], vt[:],
                                         AF.Identity)
                else:
                    nc.vector.tensor_copy(chv[:, 2 * g:2 * g + 2], vt[:])
            pj = projps.tile([E, CHUNK], f32, tag="pj", name=f"pjv{c}")
            for hb in range(HB):
                nc.tensor.matmul(
                    pj[:],
                    lhsT=wTv[:, hb],
                    rhs=chv[:, hb],
                    start=(hb == 0),
                    stop=(hb == HB - 1),
                )
            nc.vector.tensor_scalar_add(vpT[:, l0:l0 + CHUNK], pj[:], b_sb["v"][:])
            # batched vaug: 4 transposes into one PSUM tile, one copy out
            sm = smallps.tile([128, 4, E + 4], f32, tag="sm", name=f"vaugtp{c}")
            vps = sm[:].bitcast(bf16)  # [128, 4, 2*(E+4)]
            for t in range(TPC):
                jb = c * TPC + t
                nc.tensor.transpose(
                    vps[:, t, :E], vpT[:, jb * 128:(jb + 1) * 128],
                    identb[:E, :E])
            nc.vector.tensor_copy(
                vaug[:, c * TPC:(c + 1) * TPC, 0:E], vps[:, :, 0:E])

        # ---- ctx accumulation ----
        ctx_of = {}

        def emit_ctx(s, c):
            i0 = s * CHUNK
            jmax = 4 * s + 3
            if s not in ctx_of:
                ctx_of[s] = ctxps.tile([E + 1, CHUNK], f32, tag="ctx",
                                       name=f"ctx{s}")
            ctx_ps = ctx_of[s]
            jlo, jhi = 4 * c, min(4 * c + 3, jmax)
            for pt, infos in pt_info[s]:
                for j, g0, w, off in infos:
                    if not (jlo <= j <= jhi):
                        continue
                    nc.tensor.matmul(
                        ctx_ps[:, g0 - i0:g0 - i0 + w],
                        lhsT=vaug[:, j],
                        rhs=pt[:, off:off + w],
                        start=(j == 0),
                        stop=(j == jmax),
                    )

        def emit_epi(s):
            i0, i1 = s * CHUNK, (s + 1) * CHUNK
            late = s >= 2
            ctx_ps = ctx_of[s]
            ctxsb = epip.tile([E + 1, CHUNK], f32, tag="ctxsb", name=f"ctxsb{s}")
            nc.vector.tensor_copy(ctxsb[:], ctx_ps[:])
            outsb = epip.tile([128, TPC, E], f32, tag="outsb", name=f"outsb{s}")
            # batched epilogue: 4 transposes into one PSUM tile, one recip,
            # then 4 back-to-back muls
            cps = smallps.tile([128, 4, E + 4], f32, tag="sm", name=f"etp{s}")
            for t in range(TPC):
                nc.tensor.transpose(
                    cps[:, t, 0:E + 1],
                    ctxsb[:, t * 128:(t + 1) * 128],
                    identf[:E + 1, :E + 1],
                )
            rec = epip.tile([128, TPC], f32, tag="rec", name=f"rec{s}")
            nc.vector.reciprocal(rec[:], cps[:, :, E:E + 1])
            for t in range(TPC):
                nc.vector.tensor_scalar_mul(outsb[:, t, :], cps[:, t, 0:E],
                                            rec[:, t:t + 1])
            dst = out_ap[i0:i1, :].rearrange("(t p) e -> p t e", p=128)
            nc.scalar.dma_start(out=dst, in_=outsb[:])

        # ================= emission schedule =================
        for _ in range(reps):
            pt_info.clear()
            ctx_of.clear()
            emit_w_loads()
            nats = {}
            nats[("k", 0)] = emit_load("k", 0)
            nats[("q", 0)] = emit_load("q", 0)
            emit_consts()  # Pool-queue consts after first two load preps
            emit_w_prep()
            for c in range(NCHUNK):
                if c > 0:
                    nats[("k", c)] = emit_load("k", c)
                    nats[("q", c)] = emit_load("q", c)
                emit_qk_tp_proj("k", c, nats[("k", c)])
                emit_qk_tp_proj("q", c, nats[("q", c)])
                emit_scores(c)
            vnats = [emit_load("v", c) for c in range(NCHUNK)]
            # v blocks with interleaved ctx; stripe pairing keeps ctxps at 2
            # bufs without in-order-queue deadlock (epi(s) is emitted before
            # any stripe that reuses its PSUM accumulator).
            for c in range(NCHUNK):
                emit_v_chunk(c, vnats[c])
                if c == 0:
                    emit_ctx(0, 0)
                    emit_epi(0)
                    emit_ctx(1, 0)
                    emit_ctx(2, 0)
                elif c == 1:
                    emit_ctx(1, 1)
                    emit_epi(1)
                    emit_ctx(2, 1)
                    emit_ctx(3, 0)
                    emit_ctx(3, 1)
                elif c == 2:
                    emit_ctx(2, 2)
                    emit_epi(2)
                    emit_ctx(3, 2)
                else:
                    emit_ctx(3, 3)
                    emit_epi(3)

    nc.compile()
    return nc


def _get_nc(reps=1):
    key = ("nc", reps)
    if key not in _CACHE:
        _CACHE[key] = _build_nc(reps)
    return _CACHE[key]


def kernel(q, k, v, key_padding_mask=None, Wq=None, bq=None, Wk=None, bk=None,
           Wv=None, bv=None):
    from concourse.bass_utils import run_bass_kernel_spmd

    nc = _get_nc()
    f = np.float32
    shared = {
        "wq": np.ascontiguousarray(Wq, dtype=f),
        "wk": np.ascontiguousarray(Wk, dtype=f),
        "wv": np.ascontiguousarray(Wv, dtype=f),
        "bq": np.ascontiguousarray(bq, dtype=f),
        "bk": np.ascontiguousarray(bk, dtype=f),
        "bv": np.ascontiguousarray(bv, dtype=f),
    }
    in_maps = []
    for n in range(NCORES):
        m = dict(shared)
        m["q"] = np.ascontiguousarray(q[n], dtype=f)
        m["k"] = np.ascontiguousarray(k[n], dtype=f)
        m["v"] = np.ascontiguousarray(v[n], dtype=f)
        in_maps.append(m)
    res = run_bass_kernel_spmd(nc, in_maps, core_ids=list(range(NCORES)))
    out = np.stack([res.results[i]["out"] for i in range(NCORES)], axis=0)
    return out.astype(np.float32)


# revision 23
# speedup vs baseline: 1.4624x; 1.0070x over previous
"""Causal attention layer (N=8, L=2048, H=1024, E=64) on 8 TRN2 NeuronCores.

Sharding: data-parallel over batch N - one batch element per core, Q/K/V
projection weights replicated. No collectives.

Per-core pipeline (all input transposes on the PE; zero DmaTranspose -> the
DMA device only carries the cast loads + outputs, ~28us):
  1. q/k cast-loaded f32->fp8e4m3 (SWDGE) in 512-row chunks; v f32->bf16.
  2. q/k chunks transposed as bf16-VIEWED fp8 PAIRS: 16 PE transposes per
     chunk (half of plain fp8) grouped 8-per-PSUM-bank, one evacuation copy
     per bank; layout [128(h-pair), 4(b), 512(l)] where partition p of
     block b holds h = 256b+2p+{0,1} interleaved.
  3. q/k projections as fp8 DoubleRow matmuls (contract 256 h per matmul,
     0.5 cyc/row): stationary wdr[b] = [128,2(t),64(e)] (de-interleaved W,
     built once via pair-transposes of W); bias added during PSUM evac;
     qpT/kpT stored fp8e4m3 [64, 2048].
  4. Scores ALSO DoubleRow at 0.5 cyc/row via stride-0 broadcast (both
     k-tiles alias the same data => logits doubled; the 1/2 is folded into
     the exp scale). exp on ScalarE; causal mask = upper-tri multiply on
     diagonal blocks; P^T tiles persist in SBUF (bf16).
  5. v chunks: plain bf16 PE transposes (32/chunk, 8-per-bank groups) ->
     chv [128,8(hb),512], projection -> vpT bf16, PE-transposed to vaug
     [128,16,65] with a ones column (ctx matmul accumulates row-sums free).
  6. ctx^T[65, i] += vaug[j].T @ P^T[j, i] accumulated j-ordered as v chunks
     land, 2 PSUM accumulators with stripe pairing (s0,s1 then s2,s3);
     epilogue PE-transposes back, divides by row-sum, DMAs out.
Engine budget: PE ~36us (bottleneck), Act = exps + proj evacs (~26us),
DVE = transpose evacs + epilogue (~28us), Pool = SWDGE descriptor gen +
late-stage copies, DMA ~28us.
"""

import math

import numpy as np

N, L, H, E = 8, 2048, 1024, 64
NCORES = 8
CHUNK = 512
NCHUNK = L // CHUNK  # 4
TPC = CHUNK // 128  # 4 l-tiles per chunk
NBLK = L // 128  # 16 j-blocks
HB = H // 128  # 8

_CACHE = {}


def _build_nc(reps=1):
    from contextlib import ExitStack

    import concourse.mybir as mybir
    import concourse.tile as tile
    from concourse import bacc
    from concourse.masks import make_identity, make_upper_triangular
    from concourse.tile_rust import add_dep_helper

    f32 = mybir.dt.float32
    bf16 = mybir.dt.bfloat16
    fp8 = mybir.dt.float8e4
    AF = mybir.ActivationFunctionType
    DR = mybir.MatmulPerfMode.DoubleRow
    # DR stride-0 scores double the logit; fold the 1/2 into the exp scale
    scale = 1.0 / (2.0 * math.sqrt(float(L)))

    nc = bacc.Bacc("TRN2", target_bir_lowering=False, debug=False)

    q_ap = nc.dram_tensor("q", [L, H], f32, kind="ExternalInput").ap()
    k_ap = nc.dram_tensor("k", [L, H], f32, kind="ExternalInput").ap()
    v_ap = nc.dram_tensor("v", [L, H], f32, kind="ExternalInput").ap()
    wq_ap = nc.dram_tensor("wq", [E, H], f32, kind="ExternalInput").ap()
    wk_ap = nc.dram_tensor("wk", [E, H], f32, kind="ExternalInput").ap()
    wv_ap = nc.dram_tensor("wv", [E, H], f32, kind="ExternalInput").ap()
    bq_ap = nc.dram_tensor("bq", [E], f32, kind="ExternalInput").ap()
    bk_ap = nc.dram_tensor("bk", [E], f32, kind="ExternalInput").ap()
    bv_ap = nc.dram_tensor("bv", [E], f32, kind="ExternalInput").ap()
    out_ap = nc.dram_tensor("out", [L, E], f32, kind="ExternalOutput").ap()

    x_ap_of = {"q": q_ap, "k": k_ap, "v": v_ap}

    with tile.TileContext(nc) as tc, ExitStack() as ctx:
        const = ctx.enter_context(tc.tile_pool(name="const", bufs=1))
        pTsb = ctx.enter_context(tc.tile_pool(name="pTsb", bufs=1))
        nat8p = ctx.enter_context(tc.tile_pool(name="nat8", bufs=6))
        natvp = ctx.enter_context(tc.tile_pool(name="natv", bufs=4))
        trqp = ctx.enter_context(tc.tile_pool(name="trq", bufs=3))
        chvp = ctx.enter_context(tc.tile_pool(name="chv", bufs=3))
        ptp = ctx.enter_context(tc.tile_pool(name="pt", bufs=1))
        epip = ctx.enter_context(tc.tile_pool(name="epi", bufs=4))
        tpps = ctx.enter_context(tc.tile_pool(name="tpps", bufs=2, space="PSUM"))
        projps = ctx.enter_context(tc.tile_pool(name="projps", bufs=1, space="PSUM"))
        scps = ctx.enter_context(tc.tile_pool(name="scps", bufs=2, space="PSUM"))
        ctxps = ctx.enter_context(tc.tile_pool(name="ctxps", bufs=2, space="PSUM"))
        smallps = ctx.enter_context(tc.tile_pool(name="smallps", bufs=1, space="PSUM"))

        # ---- persistent tiles ----
        identf = const.tile([128, 128], f32, name="identf")
        identb = const.tile([128, 128], bf16, name="identb")
        tri_f32 = const.tile([128, 128], f32, name="tri_f32")
        tri = const.tile([128, 128], bf16, name="tri")
        wf = {}
        b_sb = {}
        for nm, bias_ap in (("q", bq_ap), ("k", bk_ap), ("v", bv_ap)):
            wf[nm] = const.tile([E, H], f32, name=f"wf_{nm}")
            b_sb[nm] = const.tile([E, 1], f32, name=f"b_{nm}")
        w8 = {nm: const.tile([E, H], fp8, name=f"w8_{nm}") for nm in ("q", "k")}
        wvb = const.tile([E, H], bf16, name="wvb")
        wdr = {nm: const.tile([128, 4, 2, E], fp8, name=f"wdr_{nm}")
               for nm in ("q", "k")}
        wTv = const.tile([128, HB, E], bf16, name="wTv")

        qpT8 = pTsb.tile([E, L], fp8, name="qpT8")
        kpT8 = pTsb.tile([E, L], fp8, name="kpT8")
        vpT = pTsb.tile([E, L], bf16, name="vpT")
        vaug = pTsb.tile([128, NBLK, E + 1], bf16, name="vaug")
        pT8_of = {"q": qpT8, "k": kpT8}

        def emit_w_loads():
            for nm, w_ap, bias_ap in (("q", wq_ap, bq_ap), ("k", wk_ap, bk_ap),
                                      ("v", wv_ap, bv_ap)):
                nc.scalar.dma_start(out=wf[nm][:], in_=w_ap)
                nc.scalar.dma_start(out=b_sb[nm][:], in_=bias_ap)

        def emit_consts():
            make_identity(nc, identf[:])
            nc.vector.tensor_copy(identb[:], identf[:])
            # tri[r, c] = 1 where c >= r (key row j <= query col i)
            make_upper_triangular(nc, tri_f32[:], val=1.0, diag=True)
            nc.vector.tensor_copy(tri[:], tri_f32[:])
            nc.vector.memset(vaug[:, :, E:E + 1], 1.0)

        def emit_w_prep():
            nc.vector.tensor_copy(w8["q"][:], wf["q"][:])
            nc.vector.tensor_copy(w8["k"][:], wf["k"][:])
            nc.vector.tensor_copy(wvb[:], wf["v"][:])
            # q/k: pair-transpose W's u16 view, de-interleave into wdr
            for nm in ("q", "k"):
                wu = w8[nm][:].bitcast(bf16)  # [64, 512]
                wps = tpps.tile([128, 2, TPC, 128], bf16, tag="tp",
                                name=f"wps_{nm}")
                for b in range(4):
                    nc.tensor.transpose(
                        wps[:, 0, b, 0:E], wu[:, b * 128:(b + 1) * 128],
                        identb[:E, :E])
                nc.vector.tensor_copy(
                    wdr[nm][:],
                    wps[:, 0, :, 0:E].bitcast(fp8).rearrange(
                        "p b (e t) -> p b t e", t=2))
            # v: plain transposes of bf16 W
            wvps = {}
            for g in range(2):
                wps = tpps.tile([128, 2, TPC, 128], bf16, tag="tp",
                                name=f"wvps{g}")
                for i in range(4):
                    hb = g * 4 + i
                    nc.tensor.transpose(
                        wps[:, i // 2, i % 2, 0:E],
                        wvb[:, hb * 128:(hb + 1) * 128], identb[:E, :E])
                nc.vector.tensor_copy(
                    wTv[:, g * 4:(g + 1) * 4].rearrange(
                        "p (a b) e -> p a b e", a=2),
                    wps[:, :, 0:2, 0:E])

        def emit_load(nm, c):
            l0 = c * CHUNK
            dtt = bf16 if nm == "v" else fp8
            pool = natvp if nm == "v" else nat8p
            nat = pool.tile([128, TPC, H], dtt, tag="nat", name=f"nat_{nm}{c}")
            src = x_ap_of[nm][l0:l0 + CHUNK, :].rearrange("(t p) h -> p t h", p=128)
            ld = nc.gpsimd.dma_start(out=nat[:].rearrange("p t h -> p (t h)"),
                                     in_=src)
            return nat, ld

        # ---- q/k chunk: pair transposes (2 groups of 8) + DR projection ----
        def emit_qk_tp_proj(nm, c, nat):
            l0 = c * CHUNK
            natu = nat[:].bitcast(bf16)  # [128, TPC, 512] u16 pairs
            trq = trqp.tile([128, 4, CHUNK], bf16, tag="trq", name=f"trq_{nm}{c}")
            for g in range(2):  # lt in {2g, 2g+1}
                tps = tpps.tile([128, 2, 4, 128], bf16, tag="tp",
                                name=f"tp_{nm}{c}{g}")
                for i in range(2):
                    lt = 2 * g + i
                    for b in range(4):
                        nc.tensor.transpose(
                            tps[:, i, b], natu[:, lt, b * 128:(b + 1) * 128],
                            identb[:])
                # [128, (lt2, b, l)] -> trq[:, b, (2g+i)*128 + l]
                nc.vector.tensor_copy(
                    trq[:, :, g * 256:(g + 1) * 256].rearrange(
                        "p b (i l) -> p i b l", i=2),
                    tps[:])
            pj = projps.tile([E, CHUNK], f32, tag="pj", name=f"pj_{nm}{c}")
            for b in range(4):
                nc.tensor.matmul(
                    pj[:],
                    lhsT=wdr[nm][:, b],
                    rhs=trq[:, b].bitcast(fp8).rearrange("p (l t) -> p t l", t=2),
                    start=(b == 0),
                    stop=(b == 3),
                    perf_mode=DR,
                )
            nc.scalar.activation(pT8_of[nm][:, l0:l0 + CHUNK], pj[:], AF.Identity,
                                 bias=b_sb[nm][:])

        # ---- scores for stripe s (all j <= 4s+3), DoubleRow stride-0 ----
        pt_info = {}

        def dr2(ap):
            return ap.rearrange("p (o l) -> p o l", o=1).broadcast_to(
                (ap.shape[0], 2, ap.shape[-1]))

        def emit_scores(s):
            i0, i1 = s * CHUNK, (s + 1) * CHUNK
            js = list(range(4 * s + 4))
            pairs = []
            while js:
                take = js[:1]
                w0 = i1 - max(i0, js[0] * 128)
                if len(js) > 1 and w0 + (i1 - max(i0, js[1] * 128)) <= 512:
                    take = js[:2]
                pairs.append(take)
                js = js[len(take):]
            infos_all = []
            for pi, take in enumerate(pairs):
                sc = scps.tile([128, 512], f32, tag="sc", name=f"sc_{s}_{pi}")
                pt = ptp.tile([128, 512], bf16, tag=f"pt_{s}_{pi}",
                              name=f"pt_{s}_{pi}")
                infos = []
                off = 0
                for j in take:
                    g0 = max(i0, j * 128)
                    w = i1 - g0
                    nc.tensor.matmul(
                        sc[:, off:off + w],
                        lhsT=dr2(kpT8[:, j * 128:(j + 1) * 128]),
                        rhs=dr2(qpT8[:, g0:g0 + w]),
                        start=True,
                        stop=True,
                        perf_mode=DR,
                    )
                    infos.append((j, g0, w, off))
                    off += w
                nc.scalar.activation(pt[:, 0:off], sc[:, 0:off], AF.Exp,
                                     scale=scale)
                for j, g0, w, off_ in infos:
                    if g0 == j * 128:  # diagonal block: causal mask
                        nc.vector.tensor_mul(
                            pt[:, off_:off_ + 128], pt[:, off_:off_ + 128], tri[:])
                infos_all.append((pt, infos))
            pt_info[s] = infos_all

        # ---- v chunk ----
        # c < NXBAR: plain PE transposes (Act/DVE have slack early).
        # c >= NXBAR: one DmaTranspose per chunk, batched AFTER all loads
        # (single DMACopy<->DmaTranspose mode transition; DMA idles then).
        NXBAR = 2
        def emit_v_chunk(c, nat, last_ld, prev_xb):
            l0 = c * CHUNK
            xb = None
            if c >= NXBAR:
                cht = chvp.tile([128, HB * TPC, 128], bf16, tag="chx",
                                name=f"chx{c}")
                xb = nc.sync.dma_start(
                    out=cht[:],
                    in_=nat[:].rearrange("p t h -> p (t h)"),
                    transpose=True,
                )
                add_dep_helper(xb.ins, last_ld.ins, sync=True,
                               reason="batch v xbars after all loads")
                # block b of cht = (lt, hb): rhs for hb = [128, lt, 128]
                chb = cht[:].rearrange("p (lt hb) l -> p lt hb l", lt=TPC, hb=HB)
                rhs_of = lambda hb: chb[:, :, hb, :]
            else:
                chv = chvp.tile([128, HB, CHUNK], bf16, tag="chv",
                                name=f"chv{c}")
                for g in range(4):  # hb in {2g, 2g+1}
                    vt = tpps.tile([128, 2, TPC, 128], bf16, tag="tp",
                                   name=f"vt{c}{g}")
                    for i in range(2):
                        hb = 2 * g + i
                        for lt in range(TPC):
                            nc.tensor.transpose(
                                vt[:, i, lt],
                                nat[:, lt, hb * 128:(hb + 1) * 128],
                                identb[:])
                    nc.vector.tensor_copy(chv[:, 2 * g:2 * g + 2], vt[:])
                rhs_of = lambda hb: chv[:, hb]
            pj = projps.tile([E, CHUNK], f32, tag="pj", name=f"pjv{c}")
            for hb in range(HB):
                nc.tensor.matmul(
                    pj[:],
                    lhsT=wTv[:, hb],
                    rhs=rhs_of(hb),
                    start=(hb == 0),
                    stop=(hb == HB - 1),
                )
            nc.vector.tensor_scalar_add(vpT[:, l0:l0 + CHUNK], pj[:], b_sb["v"][:])
            # batched vaug: 4 transposes into one PSUM tile, one copy out
            sm = smallps.tile([128, 4, E + 4], f32, tag="sm", name=f"vaugtp{c}")
            vps = sm[:].bitcast(bf16)  # [128, 4, 2*(E+4)]
            for t in range(TPC):
                jb = c * TPC + t
                nc.tensor.transpose(
                    vps[:, t, :E], vpT[:, jb * 128:(jb + 1) * 128],
                    identb[:E, :E])
            nc.vector.tensor_copy(
                vaug[:, c * TPC:(c + 1) * TPC, 0:E], vps[:, :, 0:E])
            return xb

        # ---- ctx accumulation ----
        ctx_of = {}

        def emit_ctx(s, c):
            i0 = s * CHUNK
            jmax = 4 * s + 3
            if s not in ctx_of:
                ctx_of[s] = ctxps.tile([E + 1, CHUNK], f32, tag="ctx",
                                       name=f"ctx{s}")
            ctx_ps = ctx_of[s]
            jlo, jhi = 4 * c, min(4 * c + 3, jmax)
            for pt, infos in pt_info[s]:
                for j, g0, w, off in infos:
                    if not (jlo <= j <= jhi):
                        continue
                    nc.tensor.matmul(
                        ctx_ps[:, g0 - i0:g0 - i0 + w],
                        lhsT=vaug[:, j],
                        rhs=pt[:, off:off + w],
                        start=(j == 0),
                        stop=(j == jmax),
                    )

        def emit_epi(s):
            i0, i1 = s * CHUNK, (s + 1) * CHUNK
            late = s >= 2
            ctx_ps = ctx_of[s]
            ctxsb = epip.tile([E + 1, CHUNK], f32, tag="ctxsb", name=f"ctxsb{s}")
            nc.vector.tensor_copy(ctxsb[:], ctx_ps[:])
            outsb = epip.tile([128, TPC, E], f32, tag="outsb", name=f"outsb{s}")
            # batched epilogue: 4 transposes into one PSUM tile, one recip,
            # then 4 back-to-back muls
            cps = smallps.tile([128, 4, E + 4], f32, tag="sm", name=f"etp{s}")
            for t in range(TPC):
                nc.tensor.transpose(
                    cps[:, t, 0:E + 1],
                    ctxsb[:, t * 128:(t + 1) * 128],
                    identf[:E + 1, :E + 1],
                )
            rec = epip.tile([128, TPC], f32, tag="rec", name=f"rec{s}")
            nc.vector.reciprocal(rec[:], cps[:, :, E:E + 1])
            for t in range(TPC):
                nc.vector.tensor_scalar_mul(outsb[:, t, :], cps[:, t, 0:E],
                                            rec[:, t:t + 1])
            dst = out_ap[i0:i1, :].rearrange("(t p) e -> p t e", p=128)
            nc.scalar.dma_start(out=dst, in_=outsb[:])

        # ================= emission schedule =================
        for _ in range(reps):
            pt_info.clear()
            ctx_of.clear()
            emit_w_loads()
            nats = {}
            nats[("k", 0)] = emit_load("k", 0)[0]
            nats[("q", 0)] = emit_load("q", 0)[0]
            emit_consts()  # Pool-queue consts after first two load preps
            emit_w_prep()
            for c in range(NCHUNK):
                if c > 0:
                    nats[("k", c)] = emit_load("k", c)[0]
                    nats[("q", c)] = emit_load("q", c)[0]
                emit_qk_tp_proj("k", c, nats[("k", c)])
                emit_qk_tp_proj("q", c, nats[("q", c)])
                emit_scores(c)
            vloads = [emit_load("v", c) for c in range(NCHUNK)]
            last_ld = vloads[-1][1]
            # v blocks with interleaved ctx; stripe pairing keeps ctxps at 2
            # bufs without in-order-queue deadlock (epi(s) is emitted before
            # any stripe that reuses its PSUM accumulator).
            prev_xb = None
            for c in range(NCHUNK):
                prev_xb = emit_v_chunk(c, vloads[c][0], last_ld, prev_xb)
                if c == 0:
                    emit_ctx(0, 0)
                    emit_epi(0)
                    emit_ctx(1, 0)
                    emit_ctx(2, 0)
                elif c == 1:
                    emit_ctx(1, 1)
                    emit_epi(1)
                    emit_ctx(2, 1)
                    emit_ctx(3, 0)
                    emit_ctx(3, 1)
                elif c == 2:
                    emit_ctx(2, 2)
                    emit_epi(2)
                    emit_ctx(3, 2)
                else:
                    emit_ctx(3, 3)
                    emit_epi(3)

    nc.compile()
    return nc


def _get_nc(reps=1):
    key = ("nc", reps)
    if key not in _CACHE:
        _CACHE[key] = _build_nc(reps)
    return _CACHE[key]


def kernel(q, k, v, key_padding_mask=None, Wq=None, bq=None, Wk=None, bk=None,
           Wv=None, bv=None):
    from concourse.bass_utils import run_bass_kernel_spmd

    nc = _get_nc()
    f = np.float32
    shared = {
        "wq": np.ascontiguousarray(Wq, dtype=f),
        "wk": np.ascontiguousarray(Wk, dtype=f),
        "wv": np.ascontiguousarray(Wv, dtype=f),
        "bq": np.ascontiguousarray(bq, dtype=f),
        "bk": np.ascontiguousarray(bk, dtype=f),
        "bv": np.ascontiguousarray(bv, dtype=f),
    }
    in_maps = []
    for n in range(NCORES):
        m = dict(shared)
        m["q"] = np.ascontiguousarray(q[n], dtype=f)
        m["k"] = np.ascontiguousarray(k[n], dtype=f)
        m["v"] = np.ascontiguousarray(v[n], dtype=f)
        in_maps.append(m)
    res = run_bass_kernel_spmd(nc, in_maps, core_ids=list(range(NCORES)))
    out = np.stack([res.results[i]["out"] for i in range(NCORES)], axis=0)
    return out.astype(np.float32)


# revision 30
# speedup vs baseline: 1.5868x; 1.0851x over previous
"""Causal attention layer (N=8, L=2048, H=1024, E=64) on 8 TRN2 NeuronCores.

Sharding: data-parallel over batch N - one batch element per core, Q/K/V
projection weights replicated. No collectives.

Per-core pipeline (all input transposes on the PE; zero DmaTranspose -> the
DMA device only carries the cast loads + outputs, ~28us):
  1. q/k cast-loaded f32->fp8e4m3 (SWDGE) in 512-row chunks; v f32->bf16.
  2. q/k chunks transposed as bf16-VIEWED fp8 PAIRS: 16 PE transposes per
     chunk (half of plain fp8) grouped 8-per-PSUM-bank, one evacuation copy
     per bank; layout [128(h-pair), 4(b), 512(l)] where partition p of
     block b holds h = 256b+2p+{0,1} interleaved.
  3. q/k projections as fp8 DoubleRow matmuls (contract 256 h per matmul,
     0.5 cyc/row): stationary wdr[b] = [128,2(t),64(e)] (de-interleaved W,
     built once via pair-transposes of W); bias added during PSUM evac;
     qpT/kpT stored fp8e4m3 [64, 2048].
  4. Scores ALSO DoubleRow at 0.5 cyc/row via stride-0 broadcast (both
     k-tiles alias the same data => logits doubled; the 1/2 is folded into
     the exp scale). exp on ScalarE; causal mask = upper-tri multiply on
     diagonal blocks; P^T tiles persist in SBUF (bf16).
  5. v chunks: plain bf16 PE transposes (32/chunk, 8-per-bank groups) ->
     chv [128,8(hb),512], projection -> vpT bf16, PE-transposed to vaug
     [128,16,65] with a ones column (ctx matmul accumulates row-sums free).
  6. ctx^T[65, i] += vaug[j].T @ P^T[j, i] accumulated j-ordered as v chunks
     land, 2 PSUM accumulators with stripe pairing (s0,s1 then s2,s3);
     epilogue PE-transposes back, divides by row-sum, DMAs out.
Engine budget: PE ~36us (bottleneck), Act = exps + proj evacs (~26us),
DVE = transpose evacs + epilogue (~28us), Pool = SWDGE descriptor gen +
late-stage copies, DMA ~28us.
"""

import math

import numpy as np

N, L, H, E = 8, 2048, 1024, 64
NCORES = 8
CHUNK = 512
NCHUNK = L // CHUNK  # 4
TPC = CHUNK // 128  # 4 l-tiles per chunk
NBLK = L // 128  # 16 j-blocks
HB = H // 128  # 8

_CACHE = {}


def _build_nc(reps=1):
    from contextlib import ExitStack

    import concourse.mybir as mybir
    import concourse.tile as tile
    from concourse import bacc
    from concourse.masks import make_identity, make_upper_triangular
    from concourse.tile_rust import add_dep_helper

    f32 = mybir.dt.float32
    bf16 = mybir.dt.bfloat16
    fp8 = mybir.dt.float8e4
    AF = mybir.ActivationFunctionType
    DR = mybir.MatmulPerfMode.DoubleRow
    # DR stride-0 scores double the logit; fold the 1/2 into the exp scale
    scale = 1.0 / (2.0 * math.sqrt(float(L)))

    nc = bacc.Bacc("TRN2", target_bir_lowering=False, debug=False)

    q_ap = nc.dram_tensor("q", [L, H], f32, kind="ExternalInput").ap()
    k_ap = nc.dram_tensor("k", [L, H], f32, kind="ExternalInput").ap()
    v_ap = nc.dram_tensor("v", [L, H], f32, kind="ExternalInput").ap()
    wq_ap = nc.dram_tensor("wq", [E, H], f32, kind="ExternalInput").ap()
    wk_ap = nc.dram_tensor("wk", [E, H], f32, kind="ExternalInput").ap()
    wv_ap = nc.dram_tensor("wv", [E, H], f32, kind="ExternalInput").ap()
    bq_ap = nc.dram_tensor("bq", [E], f32, kind="ExternalInput").ap()
    bk_ap = nc.dram_tensor("bk", [E], f32, kind="ExternalInput").ap()
    bv_ap = nc.dram_tensor("bv", [E], f32, kind="ExternalInput").ap()
    out_ap = nc.dram_tensor("out", [L, E], f32, kind="ExternalOutput").ap()

    x_ap_of = {"q": q_ap, "k": k_ap, "v": v_ap}

    with tile.TileContext(nc) as tc, ExitStack() as ctx:
        const = ctx.enter_context(tc.tile_pool(name="const", bufs=1))
        pTsb = ctx.enter_context(tc.tile_pool(name="pTsb", bufs=1))
        nat8p = ctx.enter_context(tc.tile_pool(name="nat8", bufs=6))
        natvp = ctx.enter_context(tc.tile_pool(name="natv", bufs=4))
        trqp = ctx.enter_context(tc.tile_pool(name="trq", bufs=3))
        chvp = ctx.enter_context(tc.tile_pool(name="chv", bufs=3))
        ptp = ctx.enter_context(tc.tile_pool(name="pt", bufs=1))
        epip = ctx.enter_context(tc.tile_pool(name="epi", bufs=4))
        tpps = ctx.enter_context(tc.tile_pool(name="tpps", bufs=2, space="PSUM"))
        projps = ctx.enter_context(tc.tile_pool(name="projps", bufs=1, space="PSUM"))
        scps = ctx.enter_context(tc.tile_pool(name="scps", bufs=2, space="PSUM"))
        ctxps = ctx.enter_context(tc.tile_pool(name="ctxps", bufs=2, space="PSUM"))
        smallps = ctx.enter_context(tc.tile_pool(name="smallps", bufs=1, space="PSUM"))

        # ---- persistent tiles ----
        identf = const.tile([128, 128], f32, name="identf")
        identb = const.tile([128, 128], bf16, name="identb")
        tri_f32 = const.tile([128, 128], f32, name="tri_f32")
        tri = const.tile([128, 128], bf16, name="tri")
        wf = {}
        b_sb = {}
        for nm, bias_ap in (("q", bq_ap), ("k", bk_ap), ("v", bv_ap)):
            wf[nm] = const.tile([E, H], f32, name=f"wf_{nm}")
            b_sb[nm] = const.tile([E, 1], f32, name=f"b_{nm}")
        w8 = {nm: const.tile([E, H], fp8, name=f"w8_{nm}") for nm in ("q", "k")}
        wvb = const.tile([E, H], bf16, name="wvb")
        wdr = {nm: const.tile([128, 4, 2, E], fp8, name=f"wdr_{nm}")
               for nm in ("q", "k")}
        wTv = const.tile([128, HB, E], bf16, name="wTv")

        qpT8 = pTsb.tile([E, L], fp8, name="qpT8")
        kpT8 = pTsb.tile([E, L], fp8, name="kpT8")
        vpT = pTsb.tile([E, L], bf16, name="vpT")
        vaug = pTsb.tile([128, NBLK, E + 1], bf16, name="vaug")
        pT8_of = {"q": qpT8, "k": kpT8}

        def emit_w_loads():
            for nm, w_ap, bias_ap in (("q", wq_ap, bq_ap), ("k", wk_ap, bk_ap),
                                      ("v", wv_ap, bv_ap)):
                nc.scalar.dma_start(out=wf[nm][:], in_=w_ap)
                nc.scalar.dma_start(out=b_sb[nm][:], in_=bias_ap)

        def emit_consts():
            make_identity(nc, identf[:])
            nc.vector.tensor_copy(identb[:], identf[:])
            # tri[r, c] = 1 where c >= r (key row j <= query col i)
            make_upper_triangular(nc, tri_f32[:], val=1.0, diag=True)
            nc.vector.tensor_copy(tri[:], tri_f32[:])
            nc.vector.memset(vaug[:, :, E:E + 1], 1.0)

        def emit_w_prep():
            nc.vector.tensor_copy(w8["q"][:], wf["q"][:])
            nc.vector.tensor_copy(w8["k"][:], wf["k"][:])
            nc.vector.tensor_copy(wvb[:], wf["v"][:])
            # q/k: pair-transpose W's u16 view, de-interleave into wdr
            for nm in ("q", "k"):
                wu = w8[nm][:].bitcast(bf16)  # [64, 512]
                wps = tpps.tile([128, 2, TPC, 128], bf16, tag="tp",
                                name=f"wps_{nm}")
                for b in range(4):
                    nc.tensor.transpose(
                        wps[:, 0, b, 0:E], wu[:, b * 128:(b + 1) * 128],
                        identb[:E, :E])
                nc.vector.tensor_copy(
                    wdr[nm][:],
                    wps[:, 0, :, 0:E].bitcast(fp8).rearrange(
                        "p b (e t) -> p b t e", t=2))
            # v: plain transposes of bf16 W
            wvps = {}
            for g in range(2):
                wps = tpps.tile([128, 2, TPC, 128], bf16, tag="tp",
                                name=f"wvps{g}")
                for i in range(4):
                    hb = g * 4 + i
                    nc.tensor.transpose(
                        wps[:, i // 2, i % 2, 0:E],
                        wvb[:, hb * 128:(hb + 1) * 128], identb[:E, :E])
                nc.vector.tensor_copy(
                    wTv[:, g * 4:(g + 1) * 4].rearrange(
                        "p (a b) e -> p a b e", a=2),
                    wps[:, :, 0:2, 0:E])

        def emit_load(nm, c):
            l0 = c * CHUNK
            dtt = bf16 if nm == "v" else fp8
            pool = natvp if nm == "v" else nat8p
            nat = pool.tile([128, TPC, H], dtt, tag="nat", name=f"nat_{nm}{c}")
            src = x_ap_of[nm][l0:l0 + CHUNK, :].rearrange("(t p) h -> p t h", p=128)
            ld = nc.gpsimd.dma_start(out=nat[:].rearrange("p t h -> p (t h)"),
                                     in_=src)
            return nat, ld

        # ---- q/k chunk: pair transposes (2 groups of 8) + DR projection ----
        def emit_qk_tp_proj(nm, c, nat):
            l0 = c * CHUNK
            natu = nat[:].bitcast(bf16)  # [128, TPC, 512] u16 pairs
            trq = trqp.tile([128, 4, CHUNK], bf16, tag="trq", name=f"trq_{nm}{c}")
            for g in range(2):  # lt in {2g, 2g+1}
                tps = tpps.tile([128, 2, 4, 128], bf16, tag="tp",
                                name=f"tp_{nm}{c}{g}")
                for i in range(2):
                    lt = 2 * g + i
                    for b in range(4):
                        nc.tensor.transpose(
                            tps[:, i, b], natu[:, lt, b * 128:(b + 1) * 128],
                            identb[:])
                # [128, (lt2, b, l)] -> trq[:, b, (2g+i)*128 + l]
                nc.vector.tensor_copy(
                    trq[:, :, g * 256:(g + 1) * 256].rearrange(
                        "p b (i l) -> p i b l", i=2),
                    tps[:])
                pump(1)
            pj = projps.tile([E, CHUNK], f32, tag="pj", name=f"pj_{nm}{c}")
            for b in range(4):
                nc.tensor.matmul(
                    pj[:],
                    lhsT=wdr[nm][:, b],
                    rhs=trq[:, b].bitcast(fp8).rearrange("p (l t) -> p t l", t=2),
                    start=(b == 0),
                    stop=(b == 3),
                    perf_mode=DR,
                )
            nc.scalar.activation(pT8_of[nm][:, l0:l0 + CHUNK], pj[:], AF.Identity,
                                 bias=b_sb[nm][:])
            pump(1)

        # ---- scores (DoubleRow stride-0), emitted lazily via pump() so the
        # in-order PE queue never head-blocks on the exp ladder ----
        pt_info = {}
        pending = []  # (s, pi, take) score pairs not yet emitted

        def dr2(ap):
            return ap.rearrange("p (o l) -> p o l", o=1).broadcast_to(
                (ap.shape[0], 2, ap.shape[-1]))

        def queue_scores(s):
            i0, i1 = s * CHUNK, (s + 1) * CHUNK
            js = list(range(4 * s + 4))
            pi = 0
            pt_info[s] = []
            while js:
                take = js[:1]
                w0 = i1 - max(i0, js[0] * 128)
                if len(js) > 1 and w0 + (i1 - max(i0, js[1] * 128)) <= 512:
                    take = js[:2]
                pending.append((s, pi, take))
                pi += 1
                js = js[len(take):]

        def pump(n):
            for _ in range(min(n, len(pending))):
                s, pi, take = pending.pop(0)
                i0, i1 = s * CHUNK, (s + 1) * CHUNK
                sc = scps.tile([128, 512], f32, tag="sc", name=f"sc_{s}_{pi}")
                pt = ptp.tile([128, 512], bf16, tag=f"pt_{s}_{pi}",
                              name=f"pt_{s}_{pi}")
                infos = []
                off = 0
                for j in take:
                    g0 = max(i0, j * 128)
                    w = i1 - g0
                    nc.tensor.matmul(
                        sc[:, off:off + w],
                        lhsT=dr2(kpT8[:, j * 128:(j + 1) * 128]),
                        rhs=dr2(qpT8[:, g0:g0 + w]),
                        start=True,
                        stop=True,
                        perf_mode=DR,
                    )
                    infos.append((j, g0, w, off))
                    off += w
                nc.scalar.activation(pt[:, 0:off], sc[:, 0:off], AF.Exp,
                                     scale=scale)
                for j, g0, w, off_ in infos:
                    if g0 == j * 128:  # diagonal block: causal mask
                        nc.vector.tensor_mul(
                            pt[:, off_:off_ + 128], pt[:, off_:off_ + 128],
                            tri[:])
                pt_info[s].append((pt, infos))

        def drain_stripe(s):
            while pending and pending[0][0] <= s:
                pump(1)

        # ---- v chunk ----
        # c < NXBAR: plain PE transposes (Act/DVE have slack early).
        # c >= NXBAR: one DmaTranspose per chunk, batched AFTER all loads
        # (single DMACopy<->DmaTranspose mode transition; DMA idles then).
        NXBAR = 2
        def emit_v_chunk(c, nat, last_ld, prev_xb):
            l0 = c * CHUNK
            xb = None
            if c >= NXBAR:
                cht = chvp.tile([128, HB * TPC, 128], bf16, tag="chx",
                                name=f"chx{c}")
                xb = nc.sync.dma_start(
                    out=cht[:],
                    in_=nat[:].rearrange("p t h -> p (t h)"),
                    transpose=True,
                )
                add_dep_helper(xb.ins, last_ld.ins, sync=True,
                               reason="batch v xbars after all loads")
                # block b of cht = (lt, hb): rhs for hb = [128, lt, 128]
                chb = cht[:].rearrange("p (lt hb) l -> p lt hb l", lt=TPC, hb=HB)
                rhs_of = lambda hb: chb[:, :, hb, :]
            else:
                chv = chvp.tile([128, HB, CHUNK], bf16, tag="chv",
                                name=f"chv{c}")
                for g in range(4):  # hb in {2g, 2g+1}
                    vt = tpps.tile([128, 2, TPC, 128], bf16, tag="tp",
                                   name=f"vt{c}{g}")
                    for i in range(2):
                        hb = 2 * g + i
                        for lt in range(TPC):
                            nc.tensor.transpose(
                                vt[:, i, lt],
                                nat[:, lt, hb * 128:(hb + 1) * 128],
                                identb[:])
                    nc.vector.tensor_copy(chv[:, 2 * g:2 * g + 2], vt[:])
                    pump(1)
                rhs_of = lambda hb: chv[:, hb]
            pj = projps.tile([E, CHUNK], f32, tag="pj", name=f"pjv{c}")
            for hb in range(HB):
                nc.tensor.matmul(
                    pj[:],
                    lhsT=wTv[:, hb],
                    rhs=rhs_of(hb),
                    start=(hb == 0),
                    stop=(hb == HB - 1),
                )
            nc.vector.tensor_scalar_add(vpT[:, l0:l0 + CHUNK], pj[:], b_sb["v"][:])
            # batched vaug: 4 transposes into one PSUM tile, one copy out
            sm = smallps.tile([128, 4, E + 4], f32, tag="sm", name=f"vaugtp{c}")
            vps = sm[:].bitcast(bf16)  # [128, 4, 2*(E+4)]
            for t in range(TPC):
                jb = c * TPC + t
                nc.tensor.transpose(
                    vps[:, t, :E], vpT[:, jb * 128:(jb + 1) * 128],
                    identb[:E, :E])
            nc.vector.tensor_copy(
                vaug[:, c * TPC:(c + 1) * TPC, 0:E], vps[:, :, 0:E])
            return xb

        # ---- ctx accumulation ----
        ctx_of = {}

        def emit_ctx(s, c):
            drain_stripe(s)
            assert not any(p[0] <= s for p in pending)
            i0 = s * CHUNK
            jmax = 4 * s + 3
            if s not in ctx_of:
                ctx_of[s] = ctxps.tile([E + 1, CHUNK], f32, tag="ctx",
                                       name=f"ctx{s}")
            ctx_ps = ctx_of[s]
            jlo, jhi = 4 * c, min(4 * c + 3, jmax)
            for pt, infos in pt_info[s]:
                for j, g0, w, off in infos:
                    if not (jlo <= j <= jhi):
                        continue
                    nc.tensor.matmul(
                        ctx_ps[:, g0 - i0:g0 - i0 + w],
                        lhsT=vaug[:, j],
                        rhs=pt[:, off:off + w],
                        start=(j == 0),
                        stop=(j == jmax),
                    )

        def emit_epi(s):
            i0, i1 = s * CHUNK, (s + 1) * CHUNK
            late = s >= 2
            ctx_ps = ctx_of[s]
            ctxsb = epip.tile([E + 1, CHUNK], f32, tag="ctxsb", name=f"ctxsb{s}")
            nc.vector.tensor_copy(ctxsb[:], ctx_ps[:])
            outsb = epip.tile([128, TPC, E], f32, tag="outsb", name=f"outsb{s}")
            # batched epilogue: 4 transposes into one PSUM tile, one recip,
            # then 4 back-to-back muls
            cps = smallps.tile([128, 4, E + 4], f32, tag="sm", name=f"etp{s}")
            for t in range(TPC):
                nc.tensor.transpose(
                    cps[:, t, 0:E + 1],
                    ctxsb[:, t * 128:(t + 1) * 128],
                    identf[:E + 1, :E + 1],
                )
            rec = epip.tile([128, TPC], f32, tag="rec", name=f"rec{s}")
            nc.vector.reciprocal(rec[:], cps[:, :, E:E + 1])
            for t in range(TPC):
                nc.vector.tensor_scalar_mul(outsb[:, t, :], cps[:, t, 0:E],
                                            rec[:, t:t + 1])
            dst = out_ap[i0:i1, :].rearrange("(t p) e -> p t e", p=128)
            nc.scalar.dma_start(out=dst, in_=outsb[:])

        # ================= emission schedule =================
        for _ in range(reps):
            pt_info.clear()
            ctx_of.clear()
            del pending[:]
            emit_w_loads()
            nats = {}
            nats[("k", 0)] = emit_load("k", 0)[0]
            nats[("q", 0)] = emit_load("q", 0)[0]
            emit_consts()  # Pool-queue consts after first two load preps
            emit_w_prep()
            for c in range(NCHUNK):
                if c > 0:
                    nats[("k", c)] = emit_load("k", c)[0]
                    nats[("q", c)] = emit_load("q", c)[0]
                emit_qk_tp_proj("k", c, nats[("k", c)])
                emit_qk_tp_proj("q", c, nats[("q", c)])
                queue_scores(c)
            vloads = [emit_load("v", c) for c in range(NCHUNK)]
            last_ld = vloads[-1][1]
            # v blocks with interleaved ctx; stripe pairing keeps ctxps at 2
            # bufs without in-order-queue deadlock (epi(s) is emitted before
            # any stripe that reuses its PSUM accumulator).
            prev_xb = None
            for c in range(NCHUNK):
                prev_xb = emit_v_chunk(c, vloads[c][0], last_ld, prev_xb)
                if c == 0:
                    emit_ctx(0, 0)
                    emit_epi(0)
                    emit_ctx(1, 0)
                    emit_ctx(2, 0)
                elif c == 1:
                    emit_ctx(1, 1)
                    emit_epi(1)
                    emit_ctx(2, 1)
                    emit_ctx(3, 0)
                    emit_ctx(3, 1)
                elif c == 2:
                    emit_ctx(2, 2)
                    emit_epi(2)
                    emit_ctx(3, 2)
                else:
                    emit_ctx(3, 3)
                    emit_epi(3)

    nc.compile()
    return nc


def _get_nc(reps=1):
    key = ("nc", reps)
    if key not in _CACHE:
        _CACHE[key] = _build_nc(reps)
    return _CACHE[key]


def kernel(q, k, v, key_padding_mask=None, Wq=None, bq=None, Wk=None, bk=None,
           Wv=None, bv=None):
    from concourse.bass_utils import run_bass_kernel_spmd

    nc = _get_nc()
    f = np.float32
    shared = {
        "wq": np.ascontiguousarray(Wq, dtype=f),
        "wk": np.ascontiguousarray(Wk, dtype=f),
        "wv": np.ascontiguousarray(Wv, dtype=f),
        "bq": np.ascontiguousarray(bq, dtype=f),
        "bk": np.ascontiguousarray(bk, dtype=f),
        "bv": np.ascontiguousarray(bv, dtype=f),
    }
    in_maps = []
    for n in range(NCORES):
        m = dict(shared)
        m["q"] = np.ascontiguousarray(q[n], dtype=f)
        m["k"] = np.ascontiguousarray(k[n], dtype=f)
        m["v"] = np.ascontiguousarray(v[n], dtype=f)
        in_maps.append(m)
    res = run_bass_kernel_spmd(nc, in_maps, core_ids=list(range(NCORES)))
    out = np.stack([res.results[i]["out"] for i in range(NCORES)], axis=0)
    return out.astype(np.float32)
